# revision 47
# baseline (speedup 1.0000x reference)
"""CrossGraphAttentionModel on 8 Trainium2 NeuronCores (Bass/Tile, SPMD).

Sharding: nodes/edges of both graphs are sharded 8 ways by dst-node range;
64-dim weights replicated. Edges are sorted by (dst_block, src_window) into
128-edge tiles with a cross-core-uniform schedule, so each GINE layer runs
entirely on the PE: x[src] is gathered with an fp8 one-hot matmul against the
AllGathered bf16 node features, the edge bias is accumulated with an identity
matmul, messages are relu'd on DVE/ACT, and scatter-adds use fp8 one-hot
matmuls into PSUM (no gpsimd DMA gathers). The node MLP stays fp32. Cross
attention runs per-head with float32r (13-mantissa-bit) score matmuls in a
head-gapped [4x32, N] layout: pass 1 computes exact row maxima ([q,k] layout,
PE + DVE/GPSIMD reduce), pass 2 recomputes scores in [k,q] with the max folded
into an appended row, exp on ACT feeds the wV matmul directly as f32r moving
data, with a ones column in V producing the softmax denominator. Pooling is a
1/count one-hot matmul, AllReduced, then the tiny output MLP.

All floating point math runs on device; the host only sorts/pads integer
index structures (one-hots are exact 0/1 patterns) and transposes/replicates
input layouts.
"""

import os

import ml_dtypes
import numpy as np

KSTAGE = int(os.environ.get("KSTAGE", "9"))

R = 8
HID = 64
B = 32
HEADS = 4
HD = 16
N_MOL, N_PROT = 2048, 4096
E_MOL, E_PROT = 32768, 131072
NC_MOL, NC_PROT = N_MOL // R, N_PROT // R              # 256, 512
NBLK_MOL, NBLK_PROT = NC_MOL // 128, NC_PROT // 128    # 2, 4
NWIN_MOL, NWIN_PROT = N_MOL // 128, N_PROT // 128      # 16, 32

FP8_ONE = 0x38  # 1.0 in float8e4m3

_CACHE = {}
last_results = None


# ----------------------------------------------------------------- host prep

def _prep_side(edge_index, eattr, N, NC, nblk, nwin):
    """Two-level (dst_block, src_window) sort with a cross-core-uniform tile
    schedule. Emits per-core fp8 one-hot gather/scatter matrices and padded
    edge features."""
    src = np.asarray(edge_index[0], np.int64)
    dst = np.asarray(edge_index[1], np.int64)
    ea = np.asarray(eattr, np.float32)
    D = ea.shape[1]
    core = dst // NC
    blk = (dst % NC) // 128
    win = src // 128
    pair = blk * nwin + win
    npair = nblk * nwin
    counts = np.zeros((R, npair), np.int64)
    np.add.at(counts, (core, pair), 1)
    tiles_pair = np.ceil(counts.max(0) / 128).astype(np.int64)
    tile_base = np.concatenate([[0], np.cumsum(tiles_pair)])
    T_total = int(tile_base[-1])
    tile_win = np.zeros(T_total, np.int64)
    tile_blk = np.zeros(T_total, np.int64)
    for p in range(npair):
        b, w = divmod(p, nwin)
        tile_win[tile_base[p]:tile_base[p + 1]] = w
        tile_blk[tile_base[p]:tile_base[p + 1]] = b
    blk_ranges = [(int(tile_base[b * nwin]), int(tile_base[(b + 1) * nwin]))
                  for b in range(nblk)]
    E_core = T_total * 128
    ohsrc = np.zeros((R, 128, T_total, 128), np.uint8)
    ohdst = np.zeros((R, 128, T_total, 128), np.uint8)
    eaT = np.zeros((R, D + 1, E_core), ml_dtypes.bfloat16)
    eaT[:, D, :] = 1.0
    for c in range(R):
        m = core == c
        s_c, d_c, p_c, ea_c = src[m], dst[m], pair[m], ea[m]
        order = np.argsort(p_c, kind="stable")
        p_s = p_c[order]
        starts = np.searchsorted(p_s, np.arange(npair))
        within = np.arange(len(p_s)) - starts[p_s]
        t = tile_base[p_s] + within // 128
        e = within % 128
        ohsrc[c, s_c[order] % 128, t, e] = FP8_ONE
        ohdst[c, e, t, (d_c[order] % NC) % 128] = FP8_ONE
        eaT[c, 0:D, t * 128 + e] = ea_c[order]
    return dict(T_total=T_total, E_core=E_core, D=D,
                tile_win=tuple(int(v) for v in tile_win),
                tile_blk=tuple(int(v) for v in tile_blk),
                blk_ranges=tuple(blk_ranges),
                ohsrc=ohsrc.view(ml_dtypes.float8_e4m3),
                ohdst=ohdst.view(ml_dtypes.float8_e4m3), eaT=eaT)


def _prep_host(inp):
    mol = _prep_side(inp["mol_edge_index"], inp["mol_eattr"],
                     N_MOL, NC_MOL, NBLK_MOL, NWIN_MOL)
    prot = _prep_side(inp["prot_edge_index"], inp["prot_eattr"],
                      N_PROT, NC_PROT, NBLK_PROT, NWIN_PROT)

    def pmat(batch, ncore):
        batch = np.asarray(batch)
        cnt = np.bincount(batch, minlength=B).astype(np.float32)
        inv = 1.0 / np.maximum(cnt, 1.0)
        m = np.zeros((R, ncore, B), np.float32)
        for c in range(R):
            sl = batch[c * ncore:(c + 1) * ncore]
            m[c, np.arange(ncore), sl] = inv[sl]
        return m

    mol_pmat = pmat(inp["mol_batch"], NC_MOL)
    prot_pmat = pmat(inp["prot_batch"], NC_PROT)

    def xt(x, ncore):
        x = np.asarray(x, np.float32)
        d = x.shape[1]
        out = np.zeros((R, d + 1, ncore), np.float32)
        for c in range(R):
            out[c, :d] = x[c * ncore:(c + 1) * ncore].T
            out[c, d] = 1.0
        return out

    mol_xT = xt(inp["mol_x"], NC_MOL)
    prot_xT = xt(inp["prot_x"], NC_PROT)

    ident = np.eye(128, dtype=np.float32)
    ident8 = ((np.eye(128) * FP8_ONE).astype(np.uint8)
              .view(ml_dtypes.float8_e4m3))

    percore = []
    for c in range(R):
        m = {
            "mol_xT": mol_xT[c], "prot_xT": prot_xT[c],
            "mol_eaT": mol["eaT"][c], "prot_eaT": prot["eaT"][c],
            "mol_ohsrc": mol["ohsrc"][c], "mol_ohdst": mol["ohdst"][c],
            "prot_ohsrc": prot["ohsrc"][c], "prot_ohdst": prot["ohdst"][c],
            "mol_pmat": mol_pmat[c], "prot_pmat": prot_pmat[c],
            "ident": ident, "ident8": ident8,
        }
        for k in ("node_lin_mol_W", "node_lin_mol_b", "node_lin_prot_W",
                  "node_lin_prot_b", "edge_lin_mol_W", "edge_lin_mol_b",
                  "edge_lin_prot_W", "edge_lin_prot_b",
                  "mol_conv_W1", "mol_conv_b1", "mol_conv_W2", "mol_conv_b2",
                  "prot_conv_W1", "prot_conv_b1", "prot_conv_W2",
                  "prot_conv_b2", "attn_mp_W", "attn_mp_b", "attn_pm_W",
                  "attn_pm_b", "fc1_W", "fc1_b", "fc2_W", "fc2_b"):
            m[k] = np.asarray(inp[k], np.float32)
        percore.append(m)

    meta = {}
    for s, d in (("mol", mol), ("prot", prot)):
        for k in ("T_total", "E_core", "D", "tile_win", "tile_blk",
                  "blk_ranges"):
            meta[f"{s}_{k}"] = d[k]
    return meta, percore


# ------------------------------------------------------------- device build

def _build(meta):
    import concourse.bacc as bacc
    import concourse.mybir as mybir
    import concourse.tile as tile

    F32 = mybir.dt.float32
    F32R = mybir.dt.float32r
    BF16 = mybir.dt.bfloat16
    FP8 = mybir.dt.float8e4
    AF = mybir.ActivationFunctionType
    ALU = mybir.AluOpType
    AX = mybir.AxisListType

    nc = bacc.Bacc("TRN2", target_bir_lowering=False, debug=False,
                   num_devices=R)

    dram = {}

    def din(name, shape, dtype=F32):
        dram[name] = nc.dram_tensor(name, list(shape), dtype,
                                    kind="ExternalInput")
        return dram[name]

    sides = {
        "mol": dict(N=N_MOL, NC=NC_MOL, nblk=NBLK_MOL, nwin=NWIN_MOL,
                    nqt=NC_MOL // 128, T=meta["mol_T_total"],
                    E=meta["mol_E_core"], D=meta["mol_D"],
                    twin=meta["mol_tile_win"], tblk=meta["mol_tile_blk"],
                    branges=meta["mol_blk_ranges"]),
        "prot": dict(N=N_PROT, NC=NC_PROT, nblk=NBLK_PROT, nwin=NWIN_PROT,
                     nqt=NC_PROT // 128, T=meta["prot_T_total"],
                     E=meta["prot_E_core"], D=meta["prot_D"],
                     twin=meta["prot_tile_win"], tblk=meta["prot_tile_blk"],
                     branges=meta["prot_blk_ranges"]),
    }

    din("mol_xT", [12, NC_MOL]); din("prot_xT", [16, NC_PROT])
    for s in sides:
        sd = sides[s]
        din(f"{s}_eaT", [11, sd["E"]], BF16)
        din(f"{s}_ohsrc", [128, sd["T"], 128], FP8)
        din(f"{s}_ohdst", [128, sd["T"], 128], FP8)
        din(f"{s}_pmat", [sd["NC"], B])
    din("ident", [128, 128]); din("ident8", [128, 128], FP8)
    din("node_lin_mol_W", [11, 64]); din("node_lin_mol_b", [64])
    din("node_lin_prot_W", [15, 64]); din("node_lin_prot_b", [64])
    din("edge_lin_mol_W", [10, 64]); din("edge_lin_mol_b", [64])
    din("edge_lin_prot_W", [10, 64]); din("edge_lin_prot_b", [64])
    for s in sides:
        din(f"{s}_conv_W1", [3, 64, 64]); din(f"{s}_conv_b1", [3, 64])
        din(f"{s}_conv_W2", [3, 64, 64]); din(f"{s}_conv_b2", [3, 64])
    din("attn_mp_W", [3, 64, 64]); din("attn_mp_b", [3, 64])
    din("attn_pm_W", [3, 64, 64]); din("attn_pm_b", [3, 64])
    din("fc1_W", [128, 64]); din("fc1_b", [64])
    din("fc2_W", [64, 1]); din("fc2_b", [1])

    out_d = nc.dram_tensor("out", [1, B], F32, kind="ExternalOutput")

    with tile.TileContext(nc) as tc:
        # ---------------- pools (SBUF release is LIFO per space)
        const = tc.alloc_tile_pool(name="const", bufs=1)
        xT_pool = tc.alloc_tile_pool(name="xT", bufs=2)
        xnf_pool = tc.alloc_tile_pool(name="xnf", bufs=2)
        ohem = tc.alloc_tile_pool(name="ohem", bufs=1)
        xsb_pool = tc.alloc_tile_pool(name="xsb", bufs=2)
        gmem = tc.alloc_tile_pool(name="gmem", bufs=1)

        def load_const(name, shape, dtype=F32, pool=None):
            t = (pool or const).tile(list(shape), dtype, name=f"c_{name}")
            nc.sync.dma_start(t[:], dram[name][:])
            return t

        ident_sb = load_const("ident", [128, 128])
        ident8_sb = load_const("ident8", [128, 128], FP8)

        def wcat(name_w, name_b, din_, dout, wslice=None):
            t = const.tile([din_ + 1, dout], F32, name=f"w_{name_w}_{wslice}")
            wsrc = dram[name_w] if wslice is None else dram[name_w][wslice]
            bsrc = dram[name_b] if wslice is None else dram[name_b][wslice]
            nc.sync.dma_start(t[0:din_, :], wsrc[:, :] if wslice is None
                              else wsrc)
            nc.sync.dma_start(t[din_:din_ + 1, :], bsrc[None, :])
            return t

        Wn = {"mol": wcat("node_lin_mol_W", "node_lin_mol_b", 11, 64),
              "prot": wcat("node_lin_prot_W", "node_lin_prot_b", 15, 64)}
        We = {"mol": wcat("edge_lin_mol_W", "edge_lin_mol_b", 10, 64),
              "prot": wcat("edge_lin_prot_W", "edge_lin_prot_b", 10, 64)}
        W1 = {s: [wcat(f"{s}_conv_W1", f"{s}_conv_b1", 64, 64, l)
                  for l in range(3)] for s in sides}
        W2 = {s: [wcat(f"{s}_conv_W2", f"{s}_conv_b2", 64, 64, l)
                  for l in range(3)] for s in sides}
        sb_xTin = {"mol": load_const("mol_xT", [12, NC_MOL]),
                   "prot": load_const("prot_xT", [16, NC_PROT])}
        sb_pmat = {}
        for s in sides:
            sd = sides[s]
            t = const.tile([128, sd["nblk"], B], F32, name=f"pmat_{s}")
            nc.sync.dma_start(
                t[:], dram[f"{s}_pmat"].rearrange("(t p) g -> p t g", p=128))
            sb_pmat[s] = t

        # one-hots + em storage (released after GINE); DMAs emitted after em
        ohsrc_sb, ohdst_sb, em_sb = {}, {}, {}
        for s in sides:
            sd = sides[s]
            T = sd["T"]
            ohsrc_sb[s] = ohem.tile([128, T, 128], FP8, name=f"ohsrc_{s}")
            ohdst_sb[s] = ohem.tile([128, T, 128], FP8, name=f"ohdst_{s}")
            em_sb[s] = ohem.tile([128, T, 64], BF16, name=f"em_{s}")

        # ---------------- DRAM internals
        dpool = tc.alloc_tile_pool(name="dram", bufs=1, space="DRAM")
        xsh_d = {s: [dpool.tile([128, sides[s]["nblk"], 64], BF16,
                                name=f"xsh_{s}_{l}") for l in range(3)]
                 for s in sides}
        xfull_d = {s: [dpool.tile([R, 128, sides[s]["nblk"], 64], BF16,
                                  addr_space="Shared", name=f"xfull_{s}_{l}")
                       for l in range(3)] for s in sides}
        xTsh_d = {s: dpool.tile([65, sides[s]["NC"]], F32R,
                                name=f"xTsh_{s}") for s in sides}
        xTfull_d = {s: dpool.tile([R, 65, sides[s]["NC"]], F32R,
                                  addr_space="Shared", name=f"xTfull_{s}")
                    for s in sides}
        zt_part_d = dpool.tile([128, B], F32, name="zt_part")
        zt_full_d = dpool.tile([128, B], F32, addr_space="Shared",
                               name="zt_full")

        alt = [0]

        def copy_alt(dst, src):
            e = (nc.vector, nc.scalar)[alt[0] % 2]
            alt[0] += 1
            if e is nc.scalar:
                nc.scalar.activation(dst, src, AF.Copy)
            else:
                nc.vector.tensor_copy(dst, src)

        # pools shared by x0 / GINE node path
        mlpps = tc.alloc_tile_pool(name="mlpps", bufs=2, space="PSUM")
        trps = tc.alloc_tile_pool(name="trps", bufs=2, space="PSUM")

        xT_cur, xnf_f32 = {}, {}
        x_sb = {}

        def push_x(s, l, xT):
            """xT fp32 -> xnf bf16 shard -> AllGather -> x_sb [128,nwin,64]."""
            sd = sides[s]
            xnf = xnf_pool.tile([128, sd["nblk"], 64], BF16,
                                name=f"xnf_{s}", tag=f"xnf_{s}")
            for b in range(sd["nblk"]):
                tp = trps.tile([128, 64], F32, name="tr_ps")
                nc.tensor.transpose(tp[:], xT[0:64, b * 128:(b + 1) * 128],
                                    ident_sb[0:64, 0:64])
                nc.vector.tensor_copy(xnf[:, b, :], tp[:])
            nc.sync.dma_start(xsh_d[s][l][:], xnf[:])
            nc.gpsimd.collective_compute(
                "AllGather", ALU.bypass, replica_groups=[list(range(R))],
                ins=[xsh_d[s][l][:].opt()], outs=[xfull_d[s][l][:].opt()])
            xs = xsb_pool.tile([128, sd["nwin"], 64], BF16,
                               name=f"xsb_{s}", tag=f"xsb_{s}")
            nblk = sd["nblk"]
            for c in range(R):
                nc.sync.dma_start(xs[:, c * nblk:(c + 1) * nblk, :],
                                  xfull_d[s][l][c])
            x_sb[s] = xs

        # initial node features first: their AllGather overlaps the em phase
        for s in ("prot", "mol"):
            sd = sides[s]
            NCs = sd["NC"]
            ps = mlpps.tile([64, 512], F32, name="mlp_ps")
            nc.tensor.matmul(ps[:, 0:NCs], Wn[s][:], sb_xTin[s][:],
                             start=True, stop=True)
            xT = xT_pool.tile([65, NCs], F32, name=f"xT_{s}", tag=f"xT_{s}")
            nc.vector.tensor_copy(xT[0:64, :], ps[:, 0:NCs])
            nc.vector.memset(xT[64:65, :], 1.0)
            xT_cur[s] = xT
            push_x(s, 0, xT)

        # ---------------- em = [eattr;1] @ [We;be]  (bf16 matmul, bf16 out)
        ea_stream = tc.alloc_tile_pool(name="ea_stream", bufs=4)
        empp = tc.alloc_tile_pool(name="empp", bufs=2, space="PSUM")
        We_bf = {}
        for s in sides:
            t = const.tile([11, 64], BF16, name=f"webf_{s}")
            nc.vector.tensor_copy(t[:], We[s][:])
            We_bf[s] = t

        for s in sides:
            sd = sides[s]
            T, D = sd["T"], sd["D"]
            CH = 16  # tiles per DMA chunk
            for t0 in range(0, T, CH):
                nt = min(CH, T - t0)
                ch = ea_stream.tile([11, CH * 128], BF16, name="ea_chunk")
                nc.sync.dma_start(
                    ch[:, 0:nt * 128],
                    dram[f"{s}_eaT"][:, t0 * 128:(t0 + nt) * 128])
                for g0 in range(0, nt, 8):
                    ng = min(8, nt - g0)
                    ps = empp.tile([128, 8, 64], F32, name="em_ps")
                    for j in range(ng):
                        nc.tensor.matmul(
                            ps[:, j, :],
                            ch[0:D + 1, (g0 + j) * 128:(g0 + j + 1) * 128],
                            We_bf[s][:], start=True, stop=True)
                    copy_alt(em_sb[s][:, t0 + g0:t0 + g0 + ng, :],
                             ps[:, 0:ng, :])
        for s in sides:
            T = sides[s]["T"]
            for kind, store in (("ohsrc", ohsrc_sb), ("ohdst", ohdst_sb)):
                step = (T + 3) // 4
                for i in range(0, T, step):
                    j = min(T, i + step)
                    nc.sync.dma_start(store[s][:, i:j, :],
                                      dram[f"{s}_{kind}"][:, i:j, :])
        ea_stream.release()
        empp.release()

        # ---------------- GINE-phase pools
        msg_pool = tc.alloc_tile_pool(name="msg", bufs=3)
        gph = tc.alloc_tile_pool(name="gph", bufs=2, space="PSUM")
        aggps = tc.alloc_tile_pool(name="aggps", bufs=2, space="PSUM")

        # GINE layers
        for l in range(3):
            for s in ("prot", "mol"):
                sd = sides[s]
                NCs, nblk, T = sd["NC"], sd["nblk"], sd["T"]
                twin, tblk, branges = sd["twin"], sd["tblk"], sd["branges"]
                xT_prev = xT_cur[s]
                hT = gmem.tile([65, NCs], F32, name=f"hT_{s}", bufs=2,
                               tag=f"hT_{s}")
                agg_tiles = {}
                for b in range(nblk):
                    t0, t1 = branges[b]
                    if t0 == t1:
                        nc.vector.tensor_copy(
                            hT[0:64, b * 128:(b + 1) * 128],
                            xT_prev[0:64, b * 128:(b + 1) * 128])
                        continue
                    agg_tiles[b] = aggps.tile([64, 128], F32, name="agg_ps")
                groups = [(g0, min(8, T - g0)) for g0 in range(0, T, 8)]
                prev = None  # (g0, ng, msg tile)

                def scatter_group(g0, ng, msgt):
                    for j in range(ng):
                        t = g0 + j
                        b = tblk[t]
                        tb0, tb1 = branges[b]
                        nc.tensor.matmul(
                            agg_tiles[b][:], msgt[:, j, :],
                            ohdst_sb[s][:, t, :],
                            start=(t == tb0), stop=(t == tb1 - 1))
                        if t == tb1 - 1:
                            nc.vector.tensor_add(
                                hT[0:64, b * 128:(b + 1) * 128],
                                xT_prev[0:64, b * 128:(b + 1) * 128],
                                agg_tiles[b][:])

                for gi, (g0, ng) in enumerate(groups):
                    psg = gph.tile([128, 8, 64], F32, name="g_ps")
                    for j in range(ng):
                        t = g0 + j
                        nc.tensor.matmul(psg[:, j, :], ohsrc_sb[s][:, t, :],
                                         x_sb[s][:, twin[t], :],
                                         start=True, stop=False)
                        nc.tensor.matmul(psg[:, j, :], ident8_sb[:],
                                         em_sb[s][:, t, :],
                                         start=False, stop=True)
                    msgt = msg_pool.tile([128, 8, 64], BF16, name="msg",
                                         tag="msg")
                    if gi % 2 == 0:
                        nc.vector.tensor_scalar_max(msgt[:, 0:ng, :],
                                                    psg[:, 0:ng, :], 0.0)
                    else:
                        nc.scalar.activation(msgt[:, 0:ng, :],
                                             psg[:, 0:ng, :], AF.Relu)
                    if prev is not None:
                        scatter_group(*prev)
                    prev = (g0, ng, msgt)
                scatter_group(*prev)
                nc.vector.memset(hT[64:65, :], 1.0)

                ps1 = mlpps.tile([64, 512], F32, name="mlp_ps")
                nc.tensor.matmul(ps1[:, 0:NCs], W1[s][l][:], hT[:],
                                 start=True, stop=True)
                r1 = gmem.tile([65, NCs], F32, name=f"r1_{s}", bufs=2,
                               tag=f"r1_{s}")
                nc.scalar.activation(r1[0:64, :], ps1[:, 0:NCs], AF.Relu)
                nc.vector.memset(r1[64:65, :], 1.0)
                ps2 = mlpps.tile([64, 512], F32, name="mlp_ps")
                nc.tensor.matmul(ps2[:, 0:NCs], W2[s][l][:], r1[:],
                                 start=True, stop=True)
                xT = xT_pool.tile([65, NCs], F32, name=f"xT_{s}",
                                  tag=f"xT_{s}")
                nc.scalar.activation(xT[0:64, :], ps2[:, 0:NCs], AF.Relu)
                nc.vector.memset(xT[64:65, :], 1.0)
                xT_cur[s] = xT
                if l < 2:
                    push_x(s, l + 1, xT)
                else:
                    # fp32 node-major copy for the attention residual
                    xnf = xnf_pool.tile([128, sd["nblk"], 64], F32,
                                        name=f"xnff_{s}", tag=f"xnff_{s}")
                    for b in range(sd["nblk"]):
                        tp = trps.tile([128, 64], F32, name="tr_ps")
                        nc.tensor.transpose(
                            tp[:], xT[0:64, b * 128:(b + 1) * 128],
                            ident_sb[0:64, 0:64])
                        nc.vector.tensor_copy(xnf[:, b, :], tp[:])
                    xnf_f32[s] = xnf
                    # f32r xT shard -> AllGather (for attention)
                    xTr = gmem.tile([65, NCs], F32R, name=f"xTr_{s}",
                                    tag=f"xTr_{s}")
                    nc.vector.tensor_copy(xTr[:], xT[:])
                    nc.sync.dma_start(xTsh_d[s][:], xTr[:])
                    nc.gpsimd.collective_compute(
                        "AllGather", ALU.bypass,
                        replica_groups=[list(range(R))],
                        ins=[xTsh_d[s][:].opt()],
                        outs=[xTfull_d[s][:].opt()])

        for p in (aggps, gph, trps, mlpps):
            p.release()
        msg_pool.release()
        gmem.release()
        xsb_pool.release()
        ohem.release()

        # ---------------- attention
        a_sb = tc.alloc_tile_pool(name="attn_sb", bufs=1)
        psPrep = tc.alloc_tile_pool(name="psPrep", bufs=2, space="PSUM")

        xT_full_r = {}
        for s in sides:
            sd = sides[s]
            t = a_sb.tile([65, sd["N"]], F32R, name=f"xTfull_{s}")
            NCs = sd["NC"]
            for c in range(R):
                nc.sync.dma_start(t[:, c * NCs:(c + 1) * NCs],
                                  xTfull_d[s][c])
            xT_full_r[s] = t

        xTq_r = {}
        for s in sides:
            sd = sides[s]
            t = a_sb.tile([65, sd["NC"]], F32R, name=f"xTq_{s}")
            nc.vector.tensor_copy(t[:], xT_cur[s][:])
            xTq_r[s] = t

        # heads packed two per tile at partition offsets 0 and 32 (base
        # partitions are restricted to {0, 32, 64})
        dirs = {"mp": ("mol", "prot"), "pm": ("prot", "mol")}
        Wq_r, Wk_r, Wv_r = {}, {}, {}
        for dirn in dirs:
            Wd, bd = dram[f"attn_{dirn}_W"], dram[f"attn_{dirn}_b"]
            wq = [a_sb.tile([65, 64], F32, name=f"wqs_{dirn}{g}")
                  for g in range(2)]
            wk = [a_sb.tile([65, 64], F32, name=f"wks_{dirn}{g}")
                  for g in range(2)]
            wv = a_sb.tile([65, HEADS * 17], F32, name=f"wvs_{dirn}")
            for g in range(2):
                nc.vector.memset(wq[g][:], 0.0)
                nc.vector.memset(wk[g][:], 0.0)
            nc.vector.memset(wv[:], 0.0)
            for h in range(HEADS):
                g, hh = divmod(h, 2)
                # head block rows: 32hh = aug (-m / ones), 32hh+1.. = Q/K
                nc.sync.dma_start(wq[g][0:64, 32 * hh + 1:32 * hh + 17],
                                  Wd[0][:, 16 * h:16 * h + 16])
                nc.sync.dma_start(wq[g][64:65, 32 * hh + 1:32 * hh + 17],
                                  bd[0][None, 16 * h:16 * h + 16])
                nc.sync.dma_start(wk[g][0:64, 32 * hh + 1:32 * hh + 17],
                                  Wd[1][:, 16 * h:16 * h + 16])
                nc.sync.dma_start(wk[g][64:65, 32 * hh + 1:32 * hh + 17],
                                  bd[1][None, 16 * h:16 * h + 16])
                nc.vector.memset(wk[g][64:65, 32 * hh:32 * hh + 1], 1.0)
                nc.sync.dma_start(wv[0:64, 17 * h:17 * h + 16],
                                  Wd[2][:, 16 * h:16 * h + 16])
                nc.sync.dma_start(wv[64:65, 17 * h:17 * h + 16],
                                  bd[2][None, 16 * h:16 * h + 16])
                nc.vector.memset(wv[64:65, 17 * h + 16:17 * h + 17], 1.0)
            Wq_r[dirn] = []
            Wk_r[dirn] = []
            for g in range(2):
                tq = a_sb.tile([65, 64], F32R, name=f"wqr_{dirn}{g}")
                nc.vector.tensor_copy(tq[:], wq[g][:])
                Wq_r[dirn].append(tq)
                tk = a_sb.tile([65, 64], F32R, name=f"wkr_{dirn}{g}")
                nc.vector.tensor_copy(tk[:], wk[g][:])
                Wk_r[dirn].append(tk)
            tv = a_sb.tile([65, HEADS * 17], F32R, name=f"wvr_{dirn}")
            nc.vector.tensor_copy(tv[:], wv[:])
            Wv_r[dirn] = tv

        QTs, KTa, Vp = {}, {}, {}
        prep_dirs = dirs if KSTAGE >= 1 else {}
        for dirn, (qs, ks) in prep_dirs.items():
            # K-side first: mp's K side (prot) is ready before mol finishes
            NCq, Nk = sides[qs]["NC"], sides[ks]["N"]
            n_k128 = Nk // 128
            KTa[dirn] = []
            for g in range(2):
                kta = a_sb.tile([64, Nk], F32R, name=f"KTa_{dirn}{g}")
                for c0 in range(0, Nk, 512):
                    psk = psPrep.tile([128, 512], F32, name="prep_ps")
                    nc.tensor.matmul(psk[0:64, :], Wk_r[dirn][g][:],
                                     xT_full_r[ks][:, c0:c0 + 512],
                                     start=True, stop=True)
                    copy_alt(kta[:, c0:c0 + 512], psk[0:64, :])
                KTa[dirn].append(kta)
            vp = a_sb.tile([128, n_k128, HEADS * 17], F32R, name=f"Vp_{dirn}")
            for kt in range(n_k128):
                psv = psPrep.tile([128, 512], F32, name="prep_ps")
                nc.tensor.matmul(psv[:, 0:HEADS * 17],
                                 xT_full_r[ks][:, kt * 128:(kt + 1) * 128],
                                 Wv_r[dirn][:], start=True, stop=True)
                copy_alt(vp[:, kt, :], psv[:, 0:HEADS * 17])
            Vp[dirn] = vp
            QTs[dirn] = []
            for g in range(2):
                psq = psPrep.tile([128, 512], F32, name="prep_ps")
                nc.tensor.matmul(psq[0:64, 0:NCq], Wq_r[dirn][g][:],
                                 xTq_r[qs][:], start=True, stop=True)
                qt_ = a_sb.tile([64, NCq], F32R, name=f"QTs_{dirn}{g}")
                nc.scalar.activation(qt_[:], psq[0:64, 0:NCq], AF.Copy,
                                     scale=0.25)
                QTs[dirn].append(qt_)
        psPrep.release()

        # interleaved pass1 (row max) / pass2 (exp + wV) chains per
        # (direction, head): chain i+2's pass1 (PE+DVE) hides under chain
        # i's pass2 (ACT-bound)
        psS1 = tc.alloc_tile_pool(name="psS1", bufs=2, space="PSUM")
        psT1 = tc.alloc_tile_pool(name="psT1", bufs=1, space="PSUM")
        psS2 = tc.alloc_tile_pool(name="psS2", bufs=2, space="PSUM")
        psOT = tc.alloc_tile_pool(name="psOT", bufs=2, space="PSUM")
        psTr = tc.alloc_tile_pool(name="psTr", bufs=1, space="PSUM")
        ex_pool = tc.alloc_tile_pool(name="expool", bufs=8)

        H_sb = {}
        for dirn, (qs, ks) in dirs.items():
            H_sb[dirn] = a_sb.tile([128, sides[qs]["NC"] // 128, 64], F32,
                                   name=f"H_{dirn}")

        def p1_emit(dirn, h):
            qs, ks = dirs[dirn]
            NCq, Nk = sides[qs]["NC"], sides[ks]["N"]
            n_qt, nch = NCq // 128, Nk // 512
            g_, hh = divmod(h, 2)
            for qt in range(n_qt):
                parts = a_sb.tile([128, 8], F32, name="mxp", bufs=4,
                                  tag="mxp")
                for g in range(nch):
                    ps = psS1.tile([128, 512], F32, name="s1_ps")
                    nc.tensor.matmul(
                        ps[:],
                        QTs[dirn][g_][32 * hh:32 * hh + 17,
                                      qt * 128:(qt + 1) * 128],
                        KTa[dirn][g_][32 * hh:32 * hh + 17,
                                      g * 512:(g + 1) * 512],
                        start=True, stop=True)
                    nc.vector.reduce_max(parts[:, g:g + 1], ps[:], axis=AX.X)
                mq = a_sb.tile([128, 1], F32, name="mq", bufs=4, tag="mq")
                nc.vector.reduce_max(mq[:], parts[:, 0:nch], axis=AX.X)
                mneg = a_sb.tile([128, 1], F32, name="mneg", bufs=4,
                                 tag="mneg")
                nc.vector.tensor_scalar_mul(mneg[:], mq[:], -1.0)
                tp = psT1.tile([1, 128], F32, name="mT_ps")
                nc.tensor.transpose(tp[:], mneg[:], ident_sb[:])
                nc.vector.tensor_copy(
                    QTs[dirn][g_][32 * hh:32 * hh + 1,
                                  qt * 128:(qt + 1) * 128], tp[:])

        def p2_emit(dirn, h):
            qs, ks = dirs[dirn]
            NCq, Nk = sides[qs]["NC"], sides[ks]["N"]
            n_qt, n_k128 = NCq // 128, Nk // 128
            g_, hh = divmod(h, 2)
            H = H_sb[dirn]
            oT = psOT.tile([17, 512], F32, name="oT_ps")
            pair = 2 if NCq <= 256 else 1  # k-chunks sharing one exp slice
            pend = []  # (kc, ex, col): wV lags one step behind s2/exp
            for kc0 in range(0, n_k128, pair):
                s2 = psS2.tile([128, 512], F32, name="s2_ps")
                cur = []
                for j in range(pair):
                    kc = kc0 + j
                    nc.tensor.matmul(
                        s2[:, j * NCq:(j + 1) * NCq],
                        KTa[dirn][g_][32 * hh:32 * hh + 17,
                                      kc * 128:(kc + 1) * 128],
                        QTs[dirn][g_][32 * hh:32 * hh + 17, :],
                        start=True, stop=True)
                ex = ex_pool.tile([128, 512], F32R, name="ex", tag="ex")
                nc.scalar.activation(ex[:, 0:pair * NCq],
                                     s2[:, 0:pair * NCq], AF.Exp)
                for j in range(pair):
                    cur.append((kc0 + j, ex, j * NCq))
                for kc, ext, col in pend:
                    nc.tensor.matmul(
                        oT[:, 0:NCq],
                        Vp[dirn][:, kc, 17 * h:17 * h + 17],
                        ext[:, col:col + NCq], start=(kc == 0), stop=False)
                pend = cur
            for kc, ext, col in pend:
                nc.tensor.matmul(
                    oT[:, 0:NCq], Vp[dirn][:, kc, 17 * h:17 * h + 17],
                    ext[:, col:col + NCq], start=(kc == 0),
                    stop=(kc == n_k128 - 1))
            oT_sb = a_sb.tile([17, NCq], F32, name="oTsb", bufs=2, tag="oTsb")
            copy_alt(oT_sb[:], oT[:, 0:NCq])
            for qt in range(n_qt):
                tp = psTr.tile([128, 17], F32, name="oTr_ps")
                nc.tensor.transpose(tp[:], oT_sb[:, qt * 128:(qt + 1) * 128],
                                    ident_sb[0:17, 0:17])
                inv = a_sb.tile([128, 1], F32, name="inv", bufs=2, tag="inv")
                nc.vector.reciprocal(inv[:], tp[:, 16:17])
                nc.vector.tensor_scalar_mul(
                    H[:, qt, 16 * h:16 * (h + 1)], tp[:, 0:16], inv[:])

        if KSTAGE >= 2:
            chains = [(d, h) for h in range(HEADS) for d in ("mp", "pm")]
            p1_emit(*chains[0])
            p1_emit(*chains[1])
            for i, c in enumerate(chains):
                if i + 2 < len(chains):
                    p1_emit(*chains[i + 2])
                p2_emit(*c)
            for dirn, (qs, ks) in dirs.items():
                nc.vector.tensor_add(H_sb[dirn][:], H_sb[dirn][:],
                                     xnf_f32[qs][:])
        else:
            for dirn, (qs, ks) in dirs.items():
                nc.vector.tensor_copy(H_sb[dirn][:], xnf_f32[qs][:])

        # ---------------- pooling + output MLP
        for dirn, qs in (("mp", "mol"), ("pm", "prot")):
            n_qt = sides[qs]["NC"] // 128
            psz = psS2.tile([128, 512], F32, name="s2_ps")
            for qt in range(n_qt):
                nc.tensor.matmul(psz[0:64, 0:B], H_sb[dirn][:, qt, :],
                                 sb_pmat[qs][:, qt, :],
                                 start=(qt == 0), stop=(qt == n_qt - 1))
            zpart = a_sb.tile([64, B], F32, name=f"zpart_{dirn}")
            nc.vector.tensor_copy(zpart[:], psz[0:64, 0:B])
            row0 = 0 if dirn == "mp" else 64
            nc.sync.dma_start(zt_part_d[row0:row0 + 64, :], zpart[:])
        nc.gpsimd.collective_compute(
            "AllReduce", ALU.add, replica_groups=[list(range(R))],
            ins=[zt_part_d[:].opt()], outs=[zt_full_d[:].opt()])
        zT = a_sb.tile([128, B], F32, name="zT")
        nc.sync.dma_start(zT[:], zt_full_d[:])

        fc1W = a_sb.tile([128, 64], F32, name="fc1W")
        nc.sync.dma_start(fc1W[:], dram["fc1_W"][:])
        fc1b = a_sb.tile([64, 1], F32, name="fc1b")
        nc.sync.dma_start(fc1b[:], dram["fc1_b"][:, None])
        fc2W = a_sb.tile([64, 1], F32, name="fc2W")
        nc.sync.dma_start(fc2W[:], dram["fc2_W"][:])
        fc2b = a_sb.tile([1, 1], F32, name="fc2b")
        nc.sync.dma_start(fc2b[:], dram["fc2_b"][:, None])

        ps = psS2.tile([128, 512], F32, name="s2_ps")
        nc.tensor.matmul(ps[0:64, 0:B], fc1W[:], zT[:], start=True, stop=True)
        h1 = a_sb.tile([65, B], F32, name="h1")
        nc.scalar.activation(h1[0:64, :], ps[0:64, 0:B], AF.Relu, bias=fc1b[:])
        ps2 = psS2.tile([128, 512], F32, name="s2_ps")
        nc.tensor.matmul(ps2[0:1, 0:B], fc2W[:], h1[0:64, :],
                         start=True, stop=True)
        osb = a_sb.tile([1, B], F32, name="osb")
        nc.scalar.activation(osb[:], ps2[0:1, 0:B], AF.Sigmoid, bias=fc2b[:])
        nc.sync.dma_start(out_d[:], osb[:])

        ex_pool.release()
        for p in (psTr, psOT, psS2, psT1, psS1):
            p.release()
        a_sb.release()
        xnf_pool.release()
        xT_pool.release()
        dpool.release()
        const.release()

    nc.compile()
    return nc


# ----------------------------------------------------------------- entry

def kernel(**inputs):
    global last_results
    meta, percore = _prep_host(inputs)
    key = (meta["mol_tile_win"], meta["mol_tile_blk"],
           meta["prot_tile_win"], meta["prot_tile_blk"])
    if key not in _CACHE:
        _CACHE[key] = _build(meta)
    nc = _CACHE[key]
    from concourse.bass_utils import run_bass_kernel_spmd
    res = run_bass_kernel_spmd(nc, percore, list(range(R)))
    last_results = res
    return np.asarray(res.results[0]["out"], np.float32).reshape(B)


# revision 50
# speedup vs baseline: 1.1081x; 1.1081x over previous
"""CrossGraphAttentionModel on 8 Trainium2 NeuronCores (Bass/Tile, SPMD).

Sharding: nodes/edges of both graphs are sharded 8 ways by dst-node range;
64-dim weights replicated. Edges are sorted by (dst_block, src_window) into
128-edge tiles with a cross-core-uniform schedule, so each GINE layer runs
entirely on the PE: x[src] is gathered with an fp8 one-hot matmul against the
AllGathered bf16 node features, the edge bias is accumulated with an identity
matmul, messages are relu'd on DVE/ACT, and scatter-adds use fp8 one-hot
matmuls into PSUM (no gpsimd DMA gathers). The node MLP stays fp32. Cross
attention runs per-head with float32r (13-mantissa-bit) score matmuls in a
head-gapped [4x32, N] layout: pass 1 computes exact row maxima ([q,k] layout,
PE + DVE/GPSIMD reduce), pass 2 recomputes scores in [k,q] with the max folded
into an appended row, exp on ACT feeds the wV matmul directly as f32r moving
data, with a ones column in V producing the softmax denominator. Pooling is a
1/count one-hot matmul, AllReduced, then the tiny output MLP.

All floating point math runs on device; the host only sorts/pads integer
index structures (one-hots are exact 0/1 patterns) and transposes/replicates
input layouts.
"""

import os

import ml_dtypes
import numpy as np

KSTAGE = int(os.environ.get("KSTAGE", "9"))

R = 8
HID = 64
B = 32
HEADS = 4
HD = 16
N_MOL, N_PROT = 2048, 4096
E_MOL, E_PROT = 32768, 131072
NC_MOL, NC_PROT = N_MOL // R, N_PROT // R              # 256, 512
NBLK_MOL, NBLK_PROT = NC_MOL // 128, NC_PROT // 128    # 2, 4
NWIN_MOL, NWIN_PROT = N_MOL // 128, N_PROT // 128      # 16, 32

FP8_ONE = 0x38  # 1.0 in float8e4m3

_CACHE = {}
last_results = None


# ----------------------------------------------------------------- host prep

def _prep_side(edge_index, eattr, N, NC, nblk, nwin):
    """Two-level (dst_block, src_window) sort with a cross-core-uniform tile
    schedule. Emits per-core fp8 one-hot gather/scatter matrices and padded
    edge features."""
    src = np.asarray(edge_index[0], np.int64)
    dst = np.asarray(edge_index[1], np.int64)
    ea = np.asarray(eattr, np.float32)
    D = ea.shape[1]
    core = dst // NC
    blk = (dst % NC) // 128
    win = src // 128
    pair = blk * nwin + win
    npair = nblk * nwin
    counts = np.zeros((R, npair), np.int64)
    np.add.at(counts, (core, pair), 1)
    tiles_pair = np.ceil(counts.max(0) / 128).astype(np.int64)
    tile_base = np.concatenate([[0], np.cumsum(tiles_pair)])
    T_total = int(tile_base[-1])
    tile_win = np.zeros(T_total, np.int64)
    tile_blk = np.zeros(T_total, np.int64)
    for p in range(npair):
        b, w = divmod(p, nwin)
        tile_win[tile_base[p]:tile_base[p + 1]] = w
        tile_blk[tile_base[p]:tile_base[p + 1]] = b
    blk_ranges = [(int(tile_base[b * nwin]), int(tile_base[(b + 1) * nwin]))
                  for b in range(nblk)]
    E_core = T_total * 128
    ohsrc = np.zeros((R, 128, T_total, 128), np.uint8)
    ohdst = np.zeros((R, 128, T_total, 128), np.uint8)
    eaT = np.zeros((R, D + 1, E_core), ml_dtypes.bfloat16)
    eaT[:, D, :] = 1.0
    for c in range(R):
        m = core == c
        s_c, d_c, p_c, ea_c = src[m], dst[m], pair[m], ea[m]
        order = np.argsort(p_c, kind="stable")
        p_s = p_c[order]
        starts = np.searchsorted(p_s, np.arange(npair))
        within = np.arange(len(p_s)) - starts[p_s]
        t = tile_base[p_s] + within // 128
        e = within % 128
        ohsrc[c, s_c[order] % 128, t, e] = FP8_ONE
        ohdst[c, e, t, (d_c[order] % NC) % 128] = FP8_ONE
        eaT[c, 0:D, t * 128 + e] = ea_c[order]
    return dict(T_total=T_total, E_core=E_core, D=D,
                tile_win=tuple(int(v) for v in tile_win),
                tile_blk=tuple(int(v) for v in tile_blk),
                blk_ranges=tuple(blk_ranges),
                ohsrc=ohsrc.view(ml_dtypes.float8_e4m3),
                ohdst=ohdst.view(ml_dtypes.float8_e4m3), eaT=eaT)


def _prep_host(inp):
    mol = _prep_side(inp["mol_edge_index"], inp["mol_eattr"],
                     N_MOL, NC_MOL, NBLK_MOL, NWIN_MOL)
    prot = _prep_side(inp["prot_edge_index"], inp["prot_eattr"],
                      N_PROT, NC_PROT, NBLK_PROT, NWIN_PROT)

    def pmat(batch, ncore):
        batch = np.asarray(batch)
        cnt = np.bincount(batch, minlength=B).astype(np.float32)
        inv = 1.0 / np.maximum(cnt, 1.0)
        m = np.zeros((R, ncore, B), np.float32)
        for c in range(R):
            sl = batch[c * ncore:(c + 1) * ncore]
            m[c, np.arange(ncore), sl] = inv[sl]
        return m

    mol_pmat = pmat(inp["mol_batch"], NC_MOL)
    prot_pmat = pmat(inp["prot_batch"], NC_PROT)

    def xt(x, ncore):
        x = np.asarray(x, np.float32)
        d = x.shape[1]
        out = np.zeros((R, d + 1, ncore), np.float32)
        for c in range(R):
            out[c, :d] = x[c * ncore:(c + 1) * ncore].T
            out[c, d] = 1.0
        return out

    mol_xT = xt(inp["mol_x"], NC_MOL)
    prot_xT = xt(inp["prot_x"], NC_PROT)

    ident = np.eye(128, dtype=np.float32)
    ident8 = ((np.eye(128) * FP8_ONE).astype(np.uint8)
              .view(ml_dtypes.float8_e4m3))

    percore = []
    for c in range(R):
        m = {
            "mol_xT": mol_xT[c], "prot_xT": prot_xT[c],
            "mol_eaT": mol["eaT"][c], "prot_eaT": prot["eaT"][c],
            "mol_ohsrc": mol["ohsrc"][c], "mol_ohdst": mol["ohdst"][c],
            "prot_ohsrc": prot["ohsrc"][c], "prot_ohdst": prot["ohdst"][c],
            "mol_pmat": mol_pmat[c], "prot_pmat": prot_pmat[c],
            "ident": ident, "ident8": ident8,
        }
        for k in ("node_lin_mol_W", "node_lin_mol_b", "node_lin_prot_W",
                  "node_lin_prot_b", "edge_lin_mol_W", "edge_lin_mol_b",
                  "edge_lin_prot_W", "edge_lin_prot_b",
                  "mol_conv_W1", "mol_conv_b1", "mol_conv_W2", "mol_conv_b2",
                  "prot_conv_W1", "prot_conv_b1", "prot_conv_W2",
                  "prot_conv_b2", "attn_mp_W", "attn_mp_b", "attn_pm_W",
                  "attn_pm_b", "fc1_W", "fc1_b", "fc2_W", "fc2_b"):
            m[k] = np.asarray(inp[k], np.float32)
        percore.append(m)

    meta = {}
    for s, d in (("mol", mol), ("prot", prot)):
        for k in ("T_total", "E_core", "D", "tile_win", "tile_blk",
                  "blk_ranges"):
            meta[f"{s}_{k}"] = d[k]
    return meta, percore


# ------------------------------------------------------------- device build

def _build(meta):
    import concourse.bacc as bacc
    import concourse.mybir as mybir
    import concourse.tile as tile

    F32 = mybir.dt.float32
    F32R = mybir.dt.float32r
    BF16 = mybir.dt.bfloat16
    FP8 = mybir.dt.float8e4
    AF = mybir.ActivationFunctionType
    ALU = mybir.AluOpType
    AX = mybir.AxisListType

    nc = bacc.Bacc("TRN2", target_bir_lowering=False, debug=False,
                   num_devices=R)

    dram = {}

    def din(name, shape, dtype=F32):
        dram[name] = nc.dram_tensor(name, list(shape), dtype,
                                    kind="ExternalInput")
        return dram[name]

    sides = {
        "mol": dict(N=N_MOL, NC=NC_MOL, nblk=NBLK_MOL, nwin=NWIN_MOL,
                    nqt=NC_MOL // 128, T=meta["mol_T_total"],
                    E=meta["mol_E_core"], D=meta["mol_D"],
                    twin=meta["mol_tile_win"], tblk=meta["mol_tile_blk"],
                    branges=meta["mol_blk_ranges"]),
        "prot": dict(N=N_PROT, NC=NC_PROT, nblk=NBLK_PROT, nwin=NWIN_PROT,
                     nqt=NC_PROT // 128, T=meta["prot_T_total"],
                     E=meta["prot_E_core"], D=meta["prot_D"],
                     twin=meta["prot_tile_win"], tblk=meta["prot_tile_blk"],
                     branges=meta["prot_blk_ranges"]),
    }

    din("mol_xT", [12, NC_MOL]); din("prot_xT", [16, NC_PROT])
    for s in sides:
        sd = sides[s]
        din(f"{s}_eaT", [11, sd["E"]], BF16)
        din(f"{s}_ohsrc", [128, sd["T"], 128], FP8)
        din(f"{s}_ohdst", [128, sd["T"], 128], FP8)
        din(f"{s}_pmat", [sd["NC"], B])
    din("ident", [128, 128]); din("ident8", [128, 128], FP8)
    din("node_lin_mol_W", [11, 64]); din("node_lin_mol_b", [64])
    din("node_lin_prot_W", [15, 64]); din("node_lin_prot_b", [64])
    din("edge_lin_mol_W", [10, 64]); din("edge_lin_mol_b", [64])
    din("edge_lin_prot_W", [10, 64]); din("edge_lin_prot_b", [64])
    for s in sides:
        din(f"{s}_conv_W1", [3, 64, 64]); din(f"{s}_conv_b1", [3, 64])
        din(f"{s}_conv_W2", [3, 64, 64]); din(f"{s}_conv_b2", [3, 64])
    din("attn_mp_W", [3, 64, 64]); din("attn_mp_b", [3, 64])
    din("attn_pm_W", [3, 64, 64]); din("attn_pm_b", [3, 64])
    din("fc1_W", [128, 64]); din("fc1_b", [64])
    din("fc2_W", [64, 1]); din("fc2_b", [1])

    out_d = nc.dram_tensor("out", [1, B], F32, kind="ExternalOutput")

    with tile.TileContext(nc) as tc:
        # ---------------- pools (SBUF release is LIFO per space)
        const = tc.alloc_tile_pool(name="const", bufs=1)
        xT_pool = tc.alloc_tile_pool(name="xT", bufs=2)
        xnf_pool = tc.alloc_tile_pool(name="xnf", bufs=2)
        ohem = tc.alloc_tile_pool(name="ohem", bufs=1)
        xsb_pool = tc.alloc_tile_pool(name="xsb", bufs=2)
        gmem = tc.alloc_tile_pool(name="gmem", bufs=1)

        def load_const(name, shape, dtype=F32, pool=None):
            t = (pool or const).tile(list(shape), dtype, name=f"c_{name}")
            nc.sync.dma_start(t[:], dram[name][:])
            return t

        ident_sb = load_const("ident", [128, 128])
        ident8_sb = load_const("ident8", [128, 128], FP8)

        def wcat(name_w, name_b, din_, dout, wslice=None):
            t = const.tile([din_ + 1, dout], F32, name=f"w_{name_w}_{wslice}")
            wsrc = dram[name_w] if wslice is None else dram[name_w][wslice]
            bsrc = dram[name_b] if wslice is None else dram[name_b][wslice]
            nc.sync.dma_start(t[0:din_, :], wsrc[:, :] if wslice is None
                              else wsrc)
            nc.sync.dma_start(t[din_:din_ + 1, :], bsrc[None, :])
            return t

        Wn = {"mol": wcat("node_lin_mol_W", "node_lin_mol_b", 11, 64),
              "prot": wcat("node_lin_prot_W", "node_lin_prot_b", 15, 64)}
        We = {"mol": wcat("edge_lin_mol_W", "edge_lin_mol_b", 10, 64),
              "prot": wcat("edge_lin_prot_W", "edge_lin_prot_b", 10, 64)}
        W1 = {s: [wcat(f"{s}_conv_W1", f"{s}_conv_b1", 64, 64, l)
                  for l in range(3)] for s in sides}
        W2 = {s: [wcat(f"{s}_conv_W2", f"{s}_conv_b2", 64, 64, l)
                  for l in range(3)] for s in sides}
        sb_xTin = {"mol": load_const("mol_xT", [12, NC_MOL]),
                   "prot": load_const("prot_xT", [16, NC_PROT])}
        sb_pmat = {}
        for s in sides:
            sd = sides[s]
            t = const.tile([128, sd["nblk"], B], F32, name=f"pmat_{s}")
            nc.sync.dma_start(
                t[:], dram[f"{s}_pmat"].rearrange("(t p) g -> p t g", p=128))
            sb_pmat[s] = t

        # one-hots + em storage (released after GINE); DMAs emitted after em
        ohsrc_sb, ohdst_sb, em_sb = {}, {}, {}
        for s in sides:
            sd = sides[s]
            T = sd["T"]
            ohsrc_sb[s] = ohem.tile([128, T, 128], FP8, name=f"ohsrc_{s}")
            ohdst_sb[s] = ohem.tile([128, T, 128], FP8, name=f"ohdst_{s}")
            em_sb[s] = ohem.tile([128, T, 64], BF16, name=f"em_{s}")

        # ---------------- DRAM internals
        dpool = tc.alloc_tile_pool(name="dram", bufs=1, space="DRAM")
        xsh_d = {s: [dpool.tile([128, sides[s]["nblk"], 64], BF16,
                                name=f"xsh_{s}_{l}") for l in range(3)]
                 for s in sides}
        xfull_d = {s: [dpool.tile([R, 128, sides[s]["nblk"], 64], BF16,
                                  addr_space="Shared", name=f"xfull_{s}_{l}")
                       for l in range(3)] for s in sides}
        xTsh_d = {s: dpool.tile([65, sides[s]["NC"]], F32R,
                                name=f"xTsh_{s}") for s in sides}
        xTfull_d = {s: dpool.tile([R, 65, sides[s]["NC"]], F32R,
                                  addr_space="Shared", name=f"xTfull_{s}")
                    for s in sides}
        zt_part_d = dpool.tile([128, B], F32, name="zt_part")
        zt_full_d = dpool.tile([128, B], F32, addr_space="Shared",
                               name="zt_full")

        # ---------------- em = [eattr;1] @ [We;be]  (bf16 matmul, bf16 out)
        ea_stream = tc.alloc_tile_pool(name="ea_stream", bufs=4)
        empp = tc.alloc_tile_pool(name="empp", bufs=2, space="PSUM")
        We_bf = {}
        for s in sides:
            t = const.tile([11, 64], BF16, name=f"webf_{s}")
            nc.vector.tensor_copy(t[:], We[s][:])
            We_bf[s] = t
        alt = [0]

        def copy_alt(dst, src):
            e = (nc.vector, nc.scalar)[alt[0] % 2]
            alt[0] += 1
            if e is nc.scalar:
                nc.scalar.activation(dst, src, AF.Copy)
            else:
                nc.vector.tensor_copy(dst, src)

        for s in sides:
            sd = sides[s]
            T, D = sd["T"], sd["D"]
            CH = 16  # tiles per DMA chunk
            for t0 in range(0, T, CH):
                nt = min(CH, T - t0)
                ch = ea_stream.tile([11, CH * 128], BF16, name="ea_chunk")
                nc.sync.dma_start(
                    ch[:, 0:nt * 128],
                    dram[f"{s}_eaT"][:, t0 * 128:(t0 + nt) * 128])
                for g0 in range(0, nt, 8):
                    ng = min(8, nt - g0)
                    ps = empp.tile([128, 8, 64], F32, name="em_ps")
                    for j in range(ng):
                        nc.tensor.matmul(
                            ps[:, j, :],
                            ch[0:D + 1, (g0 + j) * 128:(g0 + j + 1) * 128],
                            We_bf[s][:], start=True, stop=True)
                    copy_alt(em_sb[s][:, t0 + g0:t0 + g0 + ng, :],
                             ps[:, 0:ng, :])
        for s in sides:
            T = sides[s]["T"]
            for kind, store in (("ohsrc", ohsrc_sb), ("ohdst", ohdst_sb)):
                step = (T + 3) // 4
                for i in range(0, T, step):
                    j = min(T, i + step)
                    nc.sync.dma_start(store[s][:, i:j, :],
                                      dram[f"{s}_{kind}"][:, i:j, :])
        ea_stream.release()
        empp.release()

        # ---------------- GINE-phase pools
        msg_pool = tc.alloc_tile_pool(name="msg", bufs=3)
        gph = tc.alloc_tile_pool(name="gph", bufs=2, space="PSUM")
        aggps = tc.alloc_tile_pool(name="aggps", bufs=2, space="PSUM")
        mlpps = tc.alloc_tile_pool(name="mlpps", bufs=2, space="PSUM")
        trps = tc.alloc_tile_pool(name="trps", bufs=2, space="PSUM")

        xT_cur, xnf_f32 = {}, {}
        x_sb = {}

        def push_x(s, l, xT):
            """xT fp32 -> xnf bf16 shard -> AllGather -> x_sb [128,nwin,64]."""
            sd = sides[s]
            xnf = xnf_pool.tile([128, sd["nblk"], 64], BF16,
                                name=f"xnf_{s}", tag=f"xnf_{s}")
            for b in range(sd["nblk"]):
                tp = trps.tile([128, 64], F32, name="tr_ps")
                nc.tensor.transpose(tp[:], xT[0:64, b * 128:(b + 1) * 128],
                                    ident_sb[0:64, 0:64])
                nc.vector.tensor_copy(xnf[:, b, :], tp[:])
            nc.sync.dma_start(xsh_d[s][l][:], xnf[:])
            nc.gpsimd.collective_compute(
                "AllGather", ALU.bypass, replica_groups=[list(range(R))],
                ins=[xsh_d[s][l][:].opt()], outs=[xfull_d[s][l][:].opt()])
            xs = xsb_pool.tile([128, sd["nwin"], 64], BF16,
                               name=f"xsb_{s}", tag=f"xsb_{s}")
            nblk = sd["nblk"]
            for c in range(R):
                nc.sync.dma_start(xs[:, c * nblk:(c + 1) * nblk, :],
                                  xfull_d[s][l][c])
            x_sb[s] = xs

        # initial node features
        for s in ("prot", "mol"):
            sd = sides[s]
            NCs = sd["NC"]
            ps = mlpps.tile([64, 512], F32, name="mlp_ps")
            nc.tensor.matmul(ps[:, 0:NCs], Wn[s][:], sb_xTin[s][:],
                             start=True, stop=True)
            xT = xT_pool.tile([65, NCs], F32, name=f"xT_{s}", tag=f"xT_{s}")
            nc.vector.tensor_copy(xT[0:64, :], ps[:, 0:NCs])
            nc.vector.memset(xT[64:65, :], 1.0)
            xT_cur[s] = xT
            push_x(s, 0, xT)

        # GINE layers
        for l in range(3):
            for s in ("prot", "mol"):
                sd = sides[s]
                NCs, nblk, T = sd["NC"], sd["nblk"], sd["T"]
                twin, tblk, branges = sd["twin"], sd["tblk"], sd["branges"]
                xT_prev = xT_cur[s]
                hT = gmem.tile([65, NCs], F32, name=f"hT_{s}", bufs=2,
                               tag=f"hT_{s}")
                agg_tiles = {}
                for b in range(nblk):
                    t0, t1 = branges[b]
                    if t0 == t1:
                        nc.vector.tensor_copy(
                            hT[0:64, b * 128:(b + 1) * 128],
                            xT_prev[0:64, b * 128:(b + 1) * 128])
                        continue
                    agg_tiles[b] = aggps.tile([64, 128], F32, name="agg_ps")
                groups = [(g0, min(8, T - g0)) for g0 in range(0, T, 8)]
                prev = None  # (g0, ng, msg tile)

                def scatter_group(g0, ng, msgt):
                    for j in range(ng):
                        t = g0 + j
                        b = tblk[t]
                        tb0, tb1 = branges[b]
                        nc.tensor.matmul(
                            agg_tiles[b][:], msgt[:, j, :],
                            ohdst_sb[s][:, t, :],
                            start=(t == tb0), stop=(t == tb1 - 1))
                        if t == tb1 - 1:
                            nc.vector.tensor_add(
                                hT[0:64, b * 128:(b + 1) * 128],
                                xT_prev[0:64, b * 128:(b + 1) * 128],
                                agg_tiles[b][:])

                for gi, (g0, ng) in enumerate(groups):
                    psg = gph.tile([128, 8, 64], F32, name="g_ps")
                    for j in range(ng):
                        t = g0 + j
                        nc.tensor.matmul(psg[:, j, :], ohsrc_sb[s][:, t, :],
                                         x_sb[s][:, twin[t], :],
                                         start=True, stop=False)
                        nc.tensor.matmul(psg[:, j, :], ident8_sb[:],
                                         em_sb[s][:, t, :],
                                         start=False, stop=True)
                    msgt = msg_pool.tile([128, 8, 64], BF16, name="msg",
                                         tag="msg")
                    if gi % 2 == 0:
                        nc.vector.tensor_scalar_max(msgt[:, 0:ng, :],
                                                    psg[:, 0:ng, :], 0.0)
                    else:
                        nc.scalar.activation(msgt[:, 0:ng, :],
                                             psg[:, 0:ng, :], AF.Relu)
                    if prev is not None:
                        scatter_group(*prev)
                    prev = (g0, ng, msgt)
                scatter_group(*prev)
                nc.vector.memset(hT[64:65, :], 1.0)

                ps1 = mlpps.tile([64, 512], F32, name="mlp_ps")
                nc.tensor.matmul(ps1[:, 0:NCs], W1[s][l][:], hT[:],
                                 start=True, stop=True)
                r1 = gmem.tile([65, NCs], F32, name=f"r1_{s}", bufs=2,
                               tag=f"r1_{s}")
                nc.scalar.activation(r1[0:64, :], ps1[:, 0:NCs], AF.Relu)
                nc.vector.memset(r1[64:65, :], 1.0)
                ps2 = mlpps.tile([64, 512], F32, name="mlp_ps")
                nc.tensor.matmul(ps2[:, 0:NCs], W2[s][l][:], r1[:],
                                 start=True, stop=True)
                xT = xT_pool.tile([65, NCs], F32, name=f"xT_{s}",
                                  tag=f"xT_{s}")
                nc.scalar.activation(xT[0:64, :], ps2[:, 0:NCs], AF.Relu)
                nc.vector.memset(xT[64:65, :], 1.0)
                xT_cur[s] = xT
                if l < 2:
                    push_x(s, l + 1, xT)
                else:
                    # fp32 node-major copy for the attention residual
                    xnf = xnf_pool.tile([128, sd["nblk"], 64], F32,
                                        name=f"xnff_{s}", tag=f"xnff_{s}")
                    for b in range(sd["nblk"]):
                        tp = trps.tile([128, 64], F32, name="tr_ps")
                        nc.tensor.transpose(
                            tp[:], xT[0:64, b * 128:(b + 1) * 128],
                            ident_sb[0:64, 0:64])
                        nc.vector.tensor_copy(xnf[:, b, :], tp[:])
                    xnf_f32[s] = xnf
                    # f32r xT shard -> AllGather (for attention)
                    xTr = gmem.tile([65, NCs], F32R, name=f"xTr_{s}",
                                    tag=f"xTr_{s}")
                    nc.vector.tensor_copy(xTr[:], xT[:])
                    nc.sync.dma_start(xTsh_d[s][:], xTr[:])
                    nc.gpsimd.collective_compute(
                        "AllGather", ALU.bypass,
                        replica_groups=[list(range(R))],
                        ins=[xTsh_d[s][:].opt()],
                        outs=[xTfull_d[s][:].opt()])

        for p in (trps, mlpps, aggps, gph):
            p.release()
        msg_pool.release()
        gmem.release()
        xsb_pool.release()
        ohem.release()

        # ---------------- attention
        a_sb = tc.alloc_tile_pool(name="attn_sb", bufs=1)
        psPrep = tc.alloc_tile_pool(name="psPrep", bufs=2, space="PSUM")

        xT_full_r = {}
        for s in sides:
            sd = sides[s]
            t = a_sb.tile([65, sd["N"]], F32R, name=f"xTfull_{s}")
            NCs = sd["NC"]
            for c in range(R):
                nc.sync.dma_start(t[:, c * NCs:(c + 1) * NCs],
                                  xTfull_d[s][c])
            xT_full_r[s] = t

        xTq_r = {}
        for s in sides:
            sd = sides[s]
            t = a_sb.tile([65, sd["NC"]], F32R, name=f"xTq_{s}")
            nc.vector.tensor_copy(t[:], xT_cur[s][:])
            xTq_r[s] = t

        # heads packed two per tile at partition offsets 0 and 32 (base
        # partitions are restricted to {0, 32, 64})
        dirs = {"mp": ("mol", "prot"), "pm": ("prot", "mol")}
        Wq_r, Wk_r, Wv_r = {}, {}, {}
        for dirn in dirs:
            Wd, bd = dram[f"attn_{dirn}_W"], dram[f"attn_{dirn}_b"]
            wq = [a_sb.tile([65, 64], F32, name=f"wqs_{dirn}{g}")
                  for g in range(2)]
            wk = [a_sb.tile([65, 64], F32, name=f"wks_{dirn}{g}")
                  for g in range(2)]
            wv = a_sb.tile([65, HEADS * 17], F32, name=f"wvs_{dirn}")
            for g in range(2):
                nc.vector.memset(wq[g][:], 0.0)
                nc.vector.memset(wk[g][:], 0.0)
            nc.vector.memset(wv[:], 0.0)
            for h in range(HEADS):
                g, hh = divmod(h, 2)
                # head block rows: 32hh = aug (-m / ones), 32hh+1.. = Q/K
                nc.sync.dma_start(wq[g][0:64, 32 * hh + 1:32 * hh + 17],
                                  Wd[0][:, 16 * h:16 * h + 16])
                nc.sync.dma_start(wq[g][64:65, 32 * hh + 1:32 * hh + 17],
                                  bd[0][None, 16 * h:16 * h + 16])
                nc.sync.dma_start(wk[g][0:64, 32 * hh + 1:32 * hh + 17],
                                  Wd[1][:, 16 * h:16 * h + 16])
                nc.sync.dma_start(wk[g][64:65, 32 * hh + 1:32 * hh + 17],
                                  bd[1][None, 16 * h:16 * h + 16])
                nc.vector.memset(wk[g][64:65, 32 * hh:32 * hh + 1], 1.0)
                nc.sync.dma_start(wv[0:64, 17 * h:17 * h + 16],
                                  Wd[2][:, 16 * h:16 * h + 16])
                nc.sync.dma_start(wv[64:65, 17 * h:17 * h + 16],
                                  bd[2][None, 16 * h:16 * h + 16])
                nc.vector.memset(wv[64:65, 17 * h + 16:17 * h + 17], 1.0)
            Wq_r[dirn] = []
            Wk_r[dirn] = []
            for g in range(2):
                tq = a_sb.tile([65, 64], F32R, name=f"wqr_{dirn}{g}")
                nc.vector.tensor_copy(tq[:], wq[g][:])
                Wq_r[dirn].append(tq)
                tk = a_sb.tile([65, 64], F32R, name=f"wkr_{dirn}{g}")
                nc.vector.tensor_copy(tk[:], wk[g][:])
                Wk_r[dirn].append(tk)
            tv = a_sb.tile([65, HEADS * 17], F32R, name=f"wvr_{dirn}")
            nc.vector.tensor_copy(tv[:], wv[:])
            Wv_r[dirn] = tv

        QTs, KTa, Vp = {}, {}, {}
        prep_dirs = dirs if KSTAGE >= 1 else {}
        for dirn, (qs, ks) in prep_dirs.items():
            # K-side first: mp's K side (prot) is ready before mol finishes
            NCq, Nk = sides[qs]["NC"], sides[ks]["N"]
            n_k128 = Nk // 128
            KTa[dirn] = []
            for g in range(2):
                kta = a_sb.tile([64, Nk], F32R, name=f"KTa_{dirn}{g}")
                for c0 in range(0, Nk, 512):
                    psk = psPrep.tile([128, 512], F32, name="prep_ps")
                    nc.tensor.matmul(psk[0:64, :], Wk_r[dirn][g][:],
                                     xT_full_r[ks][:, c0:c0 + 512],
                                     start=True, stop=True)
                    copy_alt(kta[:, c0:c0 + 512], psk[0:64, :])
                KTa[dirn].append(kta)
            vp = a_sb.tile([128, n_k128, HEADS * 17], F32R, name=f"Vp_{dirn}")
            for kt in range(n_k128):
                psv = psPrep.tile([128, 512], F32, name="prep_ps")
                nc.tensor.matmul(psv[:, 0:HEADS * 17],
                                 xT_full_r[ks][:, kt * 128:(kt + 1) * 128],
                                 Wv_r[dirn][:], start=True, stop=True)
                copy_alt(vp[:, kt, :], psv[:, 0:HEADS * 17])
            Vp[dirn] = vp
            QTs[dirn] = []
            for g in range(2):
                psq = psPrep.tile([128, 512], F32, name="prep_ps")
                nc.tensor.matmul(psq[0:64, 0:NCq], Wq_r[dirn][g][:],
                                 xTq_r[qs][:], start=True, stop=True)
                qt_ = a_sb.tile([64, NCq], F32R, name=f"QTs_{dirn}{g}")
                nc.scalar.activation(qt_[:], psq[0:64, 0:NCq], AF.Copy,
                                     scale=0.25)
                QTs[dirn].append(qt_)
        psPrep.release()

        # interleaved pass1 (row max) / pass2 (exp + wV) chains per
        # (direction, head): chain i+2's pass1 (PE+DVE) hides under chain
        # i's pass2 (ACT-bound)
        psS1 = tc.alloc_tile_pool(name="psS1", bufs=2, space="PSUM")
        psT1 = tc.alloc_tile_pool(name="psT1", bufs=1, space="PSUM")
        psS2 = tc.alloc_tile_pool(name="psS2", bufs=2, space="PSUM")
        psOT = tc.alloc_tile_pool(name="psOT", bufs=2, space="PSUM")
        psTr = tc.alloc_tile_pool(name="psTr", bufs=1, space="PSUM")
        ex_pool = tc.alloc_tile_pool(name="expool", bufs=8)

        H_sb = {}
        for dirn, (qs, ks) in dirs.items():
            H_sb[dirn] = a_sb.tile([128, sides[qs]["NC"] // 128, 64], F32,
                                   name=f"H_{dirn}")

        def p1_emit(dirn, h):
            qs, ks = dirs[dirn]
            NCq, Nk = sides[qs]["NC"], sides[ks]["N"]
            n_qt, nch = NCq // 128, Nk // 512
            g_, hh = divmod(h, 2)
            for qt in range(n_qt):
                parts = a_sb.tile([128, 8], F32, name="mxp", bufs=4,
                                  tag="mxp")
                for g in range(nch):
                    ps = psS1.tile([128, 512], F32, name="s1_ps")
                    nc.tensor.matmul(
                        ps[:],
                        QTs[dirn][g_][32 * hh:32 * hh + 17,
                                      qt * 128:(qt + 1) * 128],
                        KTa[dirn][g_][32 * hh:32 * hh + 17,
                                      g * 512:(g + 1) * 512],
                        start=True, stop=True)
                    nc.vector.reduce_max(parts[:, g:g + 1], ps[:], axis=AX.X)
                mq = a_sb.tile([128, 1], F32, name="mq", bufs=4, tag="mq")
                nc.vector.reduce_max(mq[:], parts[:, 0:nch], axis=AX.X)
                mneg = a_sb.tile([128, 1], F32, name="mneg", bufs=4,
                                 tag="mneg")
                nc.vector.tensor_scalar_mul(mneg[:], mq[:], -1.0)
                tp = psT1.tile([1, 128], F32, name="mT_ps")
                nc.tensor.transpose(tp[:], mneg[:], ident_sb[:])
                nc.vector.tensor_copy(
                    QTs[dirn][g_][32 * hh:32 * hh + 1,
                                  qt * 128:(qt + 1) * 128], tp[:])

        def p2_emit(dirn, h):
            qs, ks = dirs[dirn]
            NCq, Nk = sides[qs]["NC"], sides[ks]["N"]
            n_qt, n_k128 = NCq // 128, Nk // 128
            g_, hh = divmod(h, 2)
            H = H_sb[dirn]
            oT = psOT.tile([17, 512], F32, name="oT_ps")
            pair = 2 if NCq <= 256 else 1  # k-chunks sharing one exp slice
            pend = []  # (kc, ex, col): wV lags one step behind s2/exp
            for kc0 in range(0, n_k128, pair):
                s2 = psS2.tile([128, 512], F32, name="s2_ps")
                cur = []
                for j in range(pair):
                    kc = kc0 + j
                    nc.tensor.matmul(
                        s2[:, j * NCq:(j + 1) * NCq],
                        KTa[dirn][g_][32 * hh:32 * hh + 17,
                                      kc * 128:(kc + 1) * 128],
                        QTs[dirn][g_][32 * hh:32 * hh + 17, :],
                        start=True, stop=True)
                ex = ex_pool.tile([128, 512], F32R, name="ex", tag="ex")
                nc.scalar.activation(ex[:, 0:pair * NCq],
                                     s2[:, 0:pair * NCq], AF.Exp)
                for j in range(pair):
                    cur.append((kc0 + j, ex, j * NCq))
                for kc, ext, col in pend:
                    nc.tensor.matmul(
                        oT[:, 0:NCq],
                        Vp[dirn][:, kc, 17 * h:17 * h + 17],
                        ext[:, col:col + NCq], start=(kc == 0), stop=False)
                pend = cur
            for kc, ext, col in pend:
                nc.tensor.matmul(
                    oT[:, 0:NCq], Vp[dirn][:, kc, 17 * h:17 * h + 17],
                    ext[:, col:col + NCq], start=(kc == 0),
                    stop=(kc == n_k128 - 1))
            oT_sb = a_sb.tile([17, NCq], F32, name="oTsb", bufs=2, tag="oTsb")
            copy_alt(oT_sb[:], oT[:, 0:NCq])
            for qt in range(n_qt):
                tp = psTr.tile([128, 17], F32, name="oTr_ps")
                nc.tensor.transpose(tp[:], oT_sb[:, qt * 128:(qt + 1) * 128],
                                    ident_sb[0:17, 0:17])
                inv = a_sb.tile([128, 1], F32, name="inv", bufs=2, tag="inv")
                nc.vector.reciprocal(inv[:], tp[:, 16:17])
                nc.vector.tensor_scalar_mul(
                    H[:, qt, 16 * h:16 * (h + 1)], tp[:, 0:16], inv[:])

        if KSTAGE >= 2:
            chains = [(d, h) for h in range(HEADS) for d in ("mp", "pm")]
            p1_emit(*chains[0])
            p1_emit(*chains[1])
            for i, c in enumerate(chains):
                if i + 2 < len(chains):
                    p1_emit(*chains[i + 2])
                p2_emit(*c)
            for dirn, (qs, ks) in dirs.items():
                nc.vector.tensor_add(H_sb[dirn][:], H_sb[dirn][:],
                                     xnf_f32[qs][:])
        else:
            for dirn, (qs, ks) in dirs.items():
                nc.vector.tensor_copy(H_sb[dirn][:], xnf_f32[qs][:])

        # ---------------- pooling + output MLP
        for dirn, qs in (("mp", "mol"), ("pm", "prot")):
            n_qt = sides[qs]["NC"] // 128
            psz = psS2.tile([128, 512], F32, name="s2_ps")
            for qt in range(n_qt):
                nc.tensor.matmul(psz[0:64, 0:B], H_sb[dirn][:, qt, :],
                                 sb_pmat[qs][:, qt, :],
                                 start=(qt == 0), stop=(qt == n_qt - 1))
            zpart = a_sb.tile([64, B], F32, name=f"zpart_{dirn}")
            nc.vector.tensor_copy(zpart[:], psz[0:64, 0:B])
            row0 = 0 if dirn == "mp" else 64
            nc.sync.dma_start(zt_part_d[row0:row0 + 64, :], zpart[:])
        nc.gpsimd.collective_compute(
            "AllReduce", ALU.add, replica_groups=[list(range(R))],
            ins=[zt_part_d[:].opt()], outs=[zt_full_d[:].opt()])
        zT = a_sb.tile([128, B], F32, name="zT")
        nc.sync.dma_start(zT[:], zt_full_d[:])

        fc1W = a_sb.tile([128, 64], F32, name="fc1W")
        nc.sync.dma_start(fc1W[:], dram["fc1_W"][:])
        fc1b = a_sb.tile([64, 1], F32, name="fc1b")
        nc.sync.dma_start(fc1b[:], dram["fc1_b"][:, None])
        fc2W = a_sb.tile([64, 1], F32, name="fc2W")
        nc.sync.dma_start(fc2W[:], dram["fc2_W"][:])
        fc2b = a_sb.tile([1, 1], F32, name="fc2b")
        nc.sync.dma_start(fc2b[:], dram["fc2_b"][:, None])

        ps = psS2.tile([128, 512], F32, name="s2_ps")
        nc.tensor.matmul(ps[0:64, 0:B], fc1W[:], zT[:], start=True, stop=True)
        h1 = a_sb.tile([65, B], F32, name="h1")
        nc.scalar.activation(h1[0:64, :], ps[0:64, 0:B], AF.Relu, bias=fc1b[:])
        ps2 = psS2.tile([128, 512], F32, name="s2_ps")
        nc.tensor.matmul(ps2[0:1, 0:B], fc2W[:], h1[0:64, :],
                         start=True, stop=True)
        osb = a_sb.tile([1, B], F32, name="osb")
        nc.scalar.activation(osb[:], ps2[0:1, 0:B], AF.Sigmoid, bias=fc2b[:])
        nc.sync.dma_start(out_d[:], osb[:])

        ex_pool.release()
        for p in (psTr, psOT, psS2, psT1, psS1):
            p.release()
        a_sb.release()
        xnf_pool.release()
        xT_pool.release()
        dpool.release()
        const.release()

    nc.compile()
    return nc


# ----------------------------------------------------------------- entry

def kernel(**inputs):
    global last_results
    meta, percore = _prep_host(inputs)
    key = (meta["mol_tile_win"], meta["mol_tile_blk"],
           meta["prot_tile_win"], meta["prot_tile_blk"])
    if key not in _CACHE:
        _CACHE[key] = _build(meta)
    nc = _CACHE[key]
    from concourse.bass_utils import run_bass_kernel_spmd
    res = run_bass_kernel_spmd(nc, percore, list(range(R)))
    last_results = res
    return np.asarray(res.results[0]["out"], np.float32).reshape(B)


# revision 55
# speedup vs baseline: 1.1512x; 1.0389x over previous
"""CrossGraphAttentionModel on 8 Trainium2 NeuronCores (Bass/Tile, SPMD).

Sharding: nodes/edges of both graphs are sharded 8 ways by dst-node range;
64-dim weights replicated. Edges are sorted by (dst_block, src_window) into
128-edge tiles with a cross-core-uniform schedule, so each GINE layer runs
entirely on the PE: x[src] is gathered with an fp8 one-hot matmul against the
AllGathered bf16 node features, the edge bias is accumulated with an identity
matmul, messages are relu'd on DVE/ACT, and scatter-adds use fp8 one-hot
matmuls into PSUM (no gpsimd DMA gathers). The node MLP stays fp32. Cross
attention runs per-head with float32r (13-mantissa-bit) score matmuls in a
head-gapped [4x32, N] layout: pass 1 computes exact row maxima ([q,k] layout,
PE + DVE/GPSIMD reduce), pass 2 recomputes scores in [k,q] with the max folded
into an appended row, exp on ACT feeds the wV matmul directly as f32r moving
data, with a ones column in V producing the softmax denominator. Pooling is a
1/count one-hot matmul, AllReduced, then the tiny output MLP.

All floating point math runs on device; the host only sorts/pads integer
index structures (one-hots are exact 0/1 patterns) and transposes/replicates
input layouts.
"""

import os

import ml_dtypes
import numpy as np

KSTAGE = int(os.environ.get("KSTAGE", "9"))

R = 8
HID = 64
B = 32
HEADS = 4
HD = 16
N_MOL, N_PROT = 2048, 4096
E_MOL, E_PROT = 32768, 131072
NC_MOL, NC_PROT = N_MOL // R, N_PROT // R              # 256, 512
NBLK_MOL, NBLK_PROT = NC_MOL // 128, NC_PROT // 128    # 2, 4
NWIN_MOL, NWIN_PROT = N_MOL // 128, N_PROT // 128      # 16, 32

FP8_ONE = 0x38  # 1.0 in float8e4m3

_CACHE = {}
last_results = None


# ----------------------------------------------------------------- host prep

def _prep_side(edge_index, eattr, N, NC, nblk, nwin):
    """Two-level (dst_block, src_window) sort with a cross-core-uniform tile
    schedule. Emits per-core fp8 one-hot gather/scatter matrices and padded
    edge features."""
    src = np.asarray(edge_index[0], np.int64)
    dst = np.asarray(edge_index[1], np.int64)
    ea = np.asarray(eattr, np.float32)
    D = ea.shape[1]
    core = dst // NC
    blk = (dst % NC) // 128
    win = src // 128
    pair = blk * nwin + win
    npair = nblk * nwin
    counts = np.zeros((R, npair), np.int64)
    np.add.at(counts, (core, pair), 1)
    tiles_pair = np.ceil(counts.max(0) / 128).astype(np.int64)
    tile_base = np.concatenate([[0], np.cumsum(tiles_pair)])
    T_total = int(tile_base[-1])
    tile_win = np.zeros(T_total, np.int64)
    tile_blk = np.zeros(T_total, np.int64)
    for p in range(npair):
        b, w = divmod(p, nwin)
        tile_win[tile_base[p]:tile_base[p + 1]] = w
        tile_blk[tile_base[p]:tile_base[p + 1]] = b
    blk_ranges = [(int(tile_base[b * nwin]), int(tile_base[(b + 1) * nwin]))
                  for b in range(nblk)]
    E_core = T_total * 128
    ohsrc = np.zeros((R, 128, T_total, 128), np.uint8)
    ohdst = np.zeros((R, 128, T_total, 128), np.uint8)
    eaT = np.zeros((R, D + 1, E_core), ml_dtypes.bfloat16)
    eaT[:, D, :] = 1.0
    for c in range(R):
        m = core == c
        s_c, d_c, p_c, ea_c = src[m], dst[m], pair[m], ea[m]
        order = np.argsort(p_c, kind="stable")
        p_s = p_c[order]
        starts = np.searchsorted(p_s, np.arange(npair))
        within = np.arange(len(p_s)) - starts[p_s]
        t = tile_base[p_s] + within // 128
        e = within % 128
        ohsrc[c, s_c[order] % 128, t, e] = FP8_ONE
        ohdst[c, e, t, (d_c[order] % NC) % 128] = FP8_ONE
        eaT[c, 0:D, t * 128 + e] = ea_c[order]
    return dict(T_total=T_total, E_core=E_core, D=D,
                tile_win=tuple(int(v) for v in tile_win),
                tile_blk=tuple(int(v) for v in tile_blk),
                blk_ranges=tuple(blk_ranges),
                ohsrc=ohsrc.view(ml_dtypes.float8_e4m3),
                ohdst=ohdst.view(ml_dtypes.float8_e4m3), eaT=eaT)


def _prep_host(inp):
    mol = _prep_side(inp["mol_edge_index"], inp["mol_eattr"],
                     N_MOL, NC_MOL, NBLK_MOL, NWIN_MOL)
    prot = _prep_side(inp["prot_edge_index"], inp["prot_eattr"],
                      N_PROT, NC_PROT, NBLK_PROT, NWIN_PROT)

    def pmat(batch, ncore):
        batch = np.asarray(batch)
        cnt = np.bincount(batch, minlength=B).astype(np.float32)
        inv = 1.0 / np.maximum(cnt, 1.0)
        m = np.zeros((R, ncore, B), np.float32)
        for c in range(R):
            sl = batch[c * ncore:(c + 1) * ncore]
            m[c, np.arange(ncore), sl] = inv[sl]
        return m

    mol_pmat = pmat(inp["mol_batch"], NC_MOL)
    prot_pmat = pmat(inp["prot_batch"], NC_PROT)

    def xt(x, ncore):
        x = np.asarray(x, np.float32)
        d = x.shape[1]
        out = np.zeros((R, d + 1, ncore), np.float32)
        for c in range(R):
            out[c, :d] = x[c * ncore:(c + 1) * ncore].T
            out[c, d] = 1.0
        return out

    mol_xT = xt(inp["mol_x"], NC_MOL)
    prot_xT = xt(inp["prot_x"], NC_PROT)

    ident = np.eye(128, dtype=np.float32)
    ident8 = ((np.eye(128) * FP8_ONE).astype(np.uint8)
              .view(ml_dtypes.float8_e4m3))

    percore = []
    for c in range(R):
        m = {
            "mol_xT": mol_xT[c], "prot_xT": prot_xT[c],
            "mol_eaT": mol["eaT"][c], "prot_eaT": prot["eaT"][c],
            "mol_ohsrc": mol["ohsrc"][c], "mol_ohdst": mol["ohdst"][c],
            "prot_ohsrc": prot["ohsrc"][c], "prot_ohdst": prot["ohdst"][c],
            "mol_pmat": mol_pmat[c], "prot_pmat": prot_pmat[c],
            "ident": ident, "ident8": ident8,
        }
        for k in ("node_lin_mol_W", "node_lin_mol_b", "node_lin_prot_W",
                  "node_lin_prot_b", "edge_lin_mol_W", "edge_lin_mol_b",
                  "edge_lin_prot_W", "edge_lin_prot_b",
                  "mol_conv_W1", "mol_conv_b1", "mol_conv_W2", "mol_conv_b2",
                  "prot_conv_W1", "prot_conv_b1", "prot_conv_W2",
                  "prot_conv_b2", "attn_mp_W", "attn_mp_b", "attn_pm_W",
                  "attn_pm_b", "fc1_W", "fc1_b", "fc2_W", "fc2_b"):
            m[k] = np.asarray(inp[k], np.float32)
        percore.append(m)

    meta = {}
    for s, d in (("mol", mol), ("prot", prot)):
        for k in ("T_total", "E_core", "D", "tile_win", "tile_blk",
                  "blk_ranges"):
            meta[f"{s}_{k}"] = d[k]
    return meta, percore


# ------------------------------------------------------------- device build

def _build(meta):
    import concourse.bacc as bacc
    import concourse.mybir as mybir
    import concourse.tile as tile

    F32 = mybir.dt.float32
    F32R = mybir.dt.float32r
    BF16 = mybir.dt.bfloat16
    FP8 = mybir.dt.float8e4
    AF = mybir.ActivationFunctionType
    ALU = mybir.AluOpType
    AX = mybir.AxisListType

    nc = bacc.Bacc("TRN2", target_bir_lowering=False, debug=False,
                   num_devices=R)

    dram = {}

    def din(name, shape, dtype=F32):
        dram[name] = nc.dram_tensor(name, list(shape), dtype,
                                    kind="ExternalInput")
        return dram[name]

    sides = {
        "mol": dict(N=N_MOL, NC=NC_MOL, nblk=NBLK_MOL, nwin=NWIN_MOL,
                    nqt=NC_MOL // 128, T=meta["mol_T_total"],
                    E=meta["mol_E_core"], D=meta["mol_D"],
                    twin=meta["mol_tile_win"], tblk=meta["mol_tile_blk"],
                    branges=meta["mol_blk_ranges"]),
        "prot": dict(N=N_PROT, NC=NC_PROT, nblk=NBLK_PROT, nwin=NWIN_PROT,
                     nqt=NC_PROT // 128, T=meta["prot_T_total"],
                     E=meta["prot_E_core"], D=meta["prot_D"],
                     twin=meta["prot_tile_win"], tblk=meta["prot_tile_blk"],
                     branges=meta["prot_blk_ranges"]),
    }

    din("mol_xT", [12, NC_MOL]); din("prot_xT", [16, NC_PROT])
    for s in sides:
        sd = sides[s]
        din(f"{s}_eaT", [11, sd["E"]], BF16)
        din(f"{s}_ohsrc", [128, sd["T"], 128], FP8)
        din(f"{s}_ohdst", [128, sd["T"], 128], FP8)
        din(f"{s}_pmat", [sd["NC"], B])
    din("ident", [128, 128]); din("ident8", [128, 128], FP8)
    din("node_lin_mol_W", [11, 64]); din("node_lin_mol_b", [64])
    din("node_lin_prot_W", [15, 64]); din("node_lin_prot_b", [64])
    din("edge_lin_mol_W", [10, 64]); din("edge_lin_mol_b", [64])
    din("edge_lin_prot_W", [10, 64]); din("edge_lin_prot_b", [64])
    for s in sides:
        din(f"{s}_conv_W1", [3, 64, 64]); din(f"{s}_conv_b1", [3, 64])
        din(f"{s}_conv_W2", [3, 64, 64]); din(f"{s}_conv_b2", [3, 64])
    din("attn_mp_W", [3, 64, 64]); din("attn_mp_b", [3, 64])
    din("attn_pm_W", [3, 64, 64]); din("attn_pm_b", [3, 64])
    din("fc1_W", [128, 64]); din("fc1_b", [64])
    din("fc2_W", [64, 1]); din("fc2_b", [1])

    out_d = nc.dram_tensor("out", [1, B], F32, kind="ExternalOutput")

    with tile.TileContext(nc) as tc:
        # ---------------- pools (SBUF release is LIFO per space)
        const = tc.alloc_tile_pool(name="const", bufs=1)
        xT_pool = tc.alloc_tile_pool(name="xT", bufs=2)
        xnf_pool = tc.alloc_tile_pool(name="xnf", bufs=2)
        ohem = tc.alloc_tile_pool(name="ohem", bufs=1)
        xsb_pool = tc.alloc_tile_pool(name="xsb", bufs=2)
        gmem = tc.alloc_tile_pool(name="gmem", bufs=1)

        def load_const(name, shape, dtype=F32, pool=None):
            t = (pool or const).tile(list(shape), dtype, name=f"c_{name}")
            nc.sync.dma_start(t[:], dram[name][:])
            return t

        ident_sb = load_const("ident", [128, 128])
        ident8_sb = load_const("ident8", [128, 128], FP8)

        def wcat(name_w, name_b, din_, dout, wslice=None):
            t = const.tile([din_ + 1, dout], F32, name=f"w_{name_w}_{wslice}")
            wsrc = dram[name_w] if wslice is None else dram[name_w][wslice]
            bsrc = dram[name_b] if wslice is None else dram[name_b][wslice]
            nc.sync.dma_start(t[0:din_, :], wsrc[:, :] if wslice is None
                              else wsrc)
            nc.sync.dma_start(t[din_:din_ + 1, :], bsrc[None, :])
            return t

        Wn = {"mol": wcat("node_lin_mol_W", "node_lin_mol_b", 11, 64),
              "prot": wcat("node_lin_prot_W", "node_lin_prot_b", 15, 64)}
        We = {"mol": wcat("edge_lin_mol_W", "edge_lin_mol_b", 10, 64),
              "prot": wcat("edge_lin_prot_W", "edge_lin_prot_b", 10, 64)}
        W1 = {s: [wcat(f"{s}_conv_W1", f"{s}_conv_b1", 64, 64, l)
                  for l in range(3)] for s in sides}
        W2 = {s: [wcat(f"{s}_conv_W2", f"{s}_conv_b2", 64, 64, l)
                  for l in range(3)] for s in sides}
        sb_xTin = {"mol": load_const("mol_xT", [12, NC_MOL]),
                   "prot": load_const("prot_xT", [16, NC_PROT])}
        sb_pmat = {}
        for s in sides:
            sd = sides[s]
            t = const.tile([128, sd["nblk"], B], F32, name=f"pmat_{s}")
            nc.sync.dma_start(
                t[:], dram[f"{s}_pmat"].rearrange("(t p) g -> p t g", p=128))
            sb_pmat[s] = t

        # one-hots + em storage (released after GINE); DMAs emitted after em
        ohsrc_sb, ohdst_sb, em_sb = {}, {}, {}
        for s in sides:
            sd = sides[s]
            T = sd["T"]
            ohsrc_sb[s] = ohem.tile([128, T, 128], FP8, name=f"ohsrc_{s}")
            ohdst_sb[s] = ohem.tile([128, T, 128], FP8, name=f"ohdst_{s}")
            em_sb[s] = ohem.tile([128, T, 64], BF16, name=f"em_{s}")

        # ---------------- DRAM internals
        dpool = tc.alloc_tile_pool(name="dram", bufs=1, space="DRAM")
        xsh_d = {s: [dpool.tile([128, sides[s]["nblk"], 64], BF16,
                                name=f"xsh_{s}_{l}") for l in range(3)]
                 for s in sides}
        xfull_d = {s: [dpool.tile([R, 128, sides[s]["nblk"], 64], BF16,
                                  addr_space="Shared", name=f"xfull_{s}_{l}")
                       for l in range(3)] for s in sides}
        xTsh_d = {s: dpool.tile([65, sides[s]["NC"]], F32R,
                                name=f"xTsh_{s}") for s in sides}
        xTfull_d = {s: dpool.tile([R, 65, sides[s]["NC"]], F32R,
                                  addr_space="Shared", name=f"xTfull_{s}")
                    for s in sides}
        zt_part_d = dpool.tile([128, B], F32, name="zt_part")
        zt_full_d = dpool.tile([128, B], F32, addr_space="Shared",
                               name="zt_full")

        # ---------------- em = [eattr;1] @ [We;be]  (bf16 matmul, bf16 out)
        ea_stream = tc.alloc_tile_pool(name="ea_stream", bufs=4)
        empp = tc.alloc_tile_pool(name="empp", bufs=2, space="PSUM")
        We_bf = {}
        for s in sides:
            t = const.tile([11, 64], BF16, name=f"webf_{s}")
            nc.vector.tensor_copy(t[:], We[s][:])
            We_bf[s] = t
        alt = [0]

        def copy_alt(dst, src):
            e = (nc.vector, nc.scalar)[alt[0] % 2]
            alt[0] += 1
            if e is nc.scalar:
                nc.scalar.activation(dst, src, AF.Copy)
            else:
                nc.vector.tensor_copy(dst, src)

        for s in sides:
            sd = sides[s]
            T, D = sd["T"], sd["D"]
            CH = 16  # tiles per DMA chunk
            for t0 in range(0, T, CH):
                nt = min(CH, T - t0)
                ch = ea_stream.tile([11, CH * 128], BF16, name="ea_chunk")
                nc.sync.dma_start(
                    ch[:, 0:nt * 128],
                    dram[f"{s}_eaT"][:, t0 * 128:(t0 + nt) * 128])
                for g0 in range(0, nt, 8):
                    ng = min(8, nt - g0)
                    ps = empp.tile([128, 8, 64], F32, name="em_ps")
                    for j in range(ng):
                        nc.tensor.matmul(
                            ps[:, j, :],
                            ch[0:D + 1, (g0 + j) * 128:(g0 + j + 1) * 128],
                            We_bf[s][:], start=True, stop=True)
                    copy_alt(em_sb[s][:, t0 + g0:t0 + g0 + ng, :],
                             ps[:, 0:ng, :])
        for s in sides:
            T = sides[s]["T"]
            for kind, store in (("ohsrc", ohsrc_sb), ("ohdst", ohdst_sb)):
                step = (T + 3) // 4
                for i in range(0, T, step):
                    j = min(T, i + step)
                    nc.sync.dma_start(store[s][:, i:j, :],
                                      dram[f"{s}_{kind}"][:, i:j, :])
        ea_stream.release()
        empp.release()

        # ---------------- GINE-phase pools
        msg_pool = tc.alloc_tile_pool(name="msg", bufs=3)
        gph = tc.alloc_tile_pool(name="gph", bufs=2, space="PSUM")
        aggps = tc.alloc_tile_pool(name="aggps", bufs=2, space="PSUM")
        mlpps = tc.alloc_tile_pool(name="mlpps", bufs=2, space="PSUM")
        trps = tc.alloc_tile_pool(name="trps", bufs=2, space="PSUM")

        xT_cur, xnf_f32 = {}, {}
        x_sb = {}

        def push_x(s, l, xT):
            """xT fp32 -> xnf bf16 shard -> AllGather -> x_sb [128,nwin,64]."""
            sd = sides[s]
            xnf = xnf_pool.tile([128, sd["nblk"], 64], BF16,
                                name=f"xnf_{s}", tag=f"xnf_{s}")
            for b in range(sd["nblk"]):
                tp = trps.tile([128, 64], F32, name="tr_ps")
                nc.tensor.transpose(tp[:], xT[0:64, b * 128:(b + 1) * 128],
                                    ident_sb[0:64, 0:64])
                nc.vector.tensor_copy(xnf[:, b, :], tp[:])
            nc.sync.dma_start(xsh_d[s][l][:], xnf[:])
            nc.gpsimd.collective_compute(
                "AllGather", ALU.bypass, replica_groups=[list(range(R))],
                ins=[xsh_d[s][l][:].opt()], outs=[xfull_d[s][l][:].opt()])
            xs = xsb_pool.tile([128, sd["nwin"], 64], BF16,
                               name=f"xsb_{s}", tag=f"xsb_{s}")
            nblk = sd["nblk"]
            for c in range(R):
                nc.sync.dma_start(xs[:, c * nblk:(c + 1) * nblk, :],
                                  xfull_d[s][l][c])
            x_sb[s] = xs

        # initial node features
        for s in ("prot", "mol"):
            sd = sides[s]
            NCs = sd["NC"]
            ps = mlpps.tile([64, 512], F32, name="mlp_ps")
            nc.tensor.matmul(ps[:, 0:NCs], Wn[s][:], sb_xTin[s][:],
                             start=True, stop=True)
            xT = xT_pool.tile([65, NCs], F32, name=f"xT_{s}", tag=f"xT_{s}")
            nc.vector.tensor_copy(xT[0:64, :], ps[:, 0:NCs])
            nc.vector.memset(xT[64:65, :], 1.0)
            xT_cur[s] = xT
            push_x(s, 0, xT)

        # GINE layers; in the last layer mol goes first so its xT
        # AllGather overlaps prot's compute and attention prep starts sooner
        for l in range(3):
            for s in (("mol", "prot") if l == 2 else ("prot", "mol")):
                sd = sides[s]
                NCs, nblk, T = sd["NC"], sd["nblk"], sd["T"]
                twin, tblk, branges = sd["twin"], sd["tblk"], sd["branges"]
                xT_prev = xT_cur[s]
                hT = gmem.tile([65, NCs], F32, name=f"hT_{s}", bufs=2,
                               tag=f"hT_{s}")
                agg_tiles = {}
                for b in range(nblk):
                    t0, t1 = branges[b]
                    if t0 == t1:
                        nc.vector.tensor_copy(
                            hT[0:64, b * 128:(b + 1) * 128],
                            xT_prev[0:64, b * 128:(b + 1) * 128])
                        continue
                    agg_tiles[b] = aggps.tile([64, 128], F32, name="agg_ps")
                groups = [(g0, min(8, T - g0)) for g0 in range(0, T, 8)]
                prev = None  # (g0, ng, msg tile)

                def scatter_group(g0, ng, msgt):
                    for j in range(ng):
                        t = g0 + j
                        b = tblk[t]
                        tb0, tb1 = branges[b]
                        nc.tensor.matmul(
                            agg_tiles[b][:], msgt[:, j, :],
                            ohdst_sb[s][:, t, :],
                            start=(t == tb0), stop=(t == tb1 - 1))
                        if t == tb1 - 1:
                            nc.vector.tensor_add(
                                hT[0:64, b * 128:(b + 1) * 128],
                                xT_prev[0:64, b * 128:(b + 1) * 128],
                                agg_tiles[b][:])

                for gi, (g0, ng) in enumerate(groups):
                    psg = gph.tile([128, 8, 64], F32, name="g_ps")
                    for j in range(ng):
                        t = g0 + j
                        nc.tensor.matmul(psg[:, j, :], ohsrc_sb[s][:, t, :],
                                         x_sb[s][:, twin[t], :],
                                         start=True, stop=False)
                        nc.tensor.matmul(psg[:, j, :], ident8_sb[:],
                                         em_sb[s][:, t, :],
                                         start=False, stop=True)
                    msgt = msg_pool.tile([128, 8, 64], BF16, name="msg",
                                         tag="msg")
                    if gi % 2 == 0:
                        nc.vector.tensor_scalar_max(msgt[:, 0:ng, :],
                                                    psg[:, 0:ng, :], 0.0)
                    else:
                        nc.scalar.activation(msgt[:, 0:ng, :],
                                             psg[:, 0:ng, :], AF.Relu)
                    if prev is not None:
                        scatter_group(*prev)
                    prev = (g0, ng, msgt)
                scatter_group(*prev)
                nc.vector.memset(hT[64:65, :], 1.0)

                ps1 = mlpps.tile([64, 512], F32, name="mlp_ps")
                nc.tensor.matmul(ps1[:, 0:NCs], W1[s][l][:], hT[:],
                                 start=True, stop=True)
                r1 = gmem.tile([65, NCs], F32, name=f"r1_{s}", bufs=2,
                               tag=f"r1_{s}")
                nc.scalar.activation(r1[0:64, :], ps1[:, 0:NCs], AF.Relu)
                nc.vector.memset(r1[64:65, :], 1.0)
                ps2 = mlpps.tile([64, 512], F32, name="mlp_ps")
                nc.tensor.matmul(ps2[:, 0:NCs], W2[s][l][:], r1[:],
                                 start=True, stop=True)
                xT = xT_pool.tile([65, NCs], F32, name=f"xT_{s}",
                                  tag=f"xT_{s}")
                nc.scalar.activation(xT[0:64, :], ps2[:, 0:NCs], AF.Relu)
                nc.vector.memset(xT[64:65, :], 1.0)
                xT_cur[s] = xT
                if l < 2:
                    push_x(s, l + 1, xT)
                else:
                    # fp32 node-major copy for the attention residual
                    xnf = xnf_pool.tile([128, sd["nblk"], 64], F32,
                                        name=f"xnff_{s}", tag=f"xnff_{s}")
                    for b in range(sd["nblk"]):
                        tp = trps.tile([128, 64], F32, name="tr_ps")
                        nc.tensor.transpose(
                            tp[:], xT[0:64, b * 128:(b + 1) * 128],
                            ident_sb[0:64, 0:64])
                        nc.vector.tensor_copy(xnf[:, b, :], tp[:])
                    xnf_f32[s] = xnf
                    # f32r xT shard -> AllGather (for attention)
                    xTr = gmem.tile([65, NCs], F32R, name=f"xTr_{s}",
                                    tag=f"xTr_{s}")
                    nc.vector.tensor_copy(xTr[:], xT[:])
                    nc.sync.dma_start(xTsh_d[s][:], xTr[:])
                    nc.gpsimd.collective_compute(
                        "AllGather", ALU.bypass,
                        replica_groups=[list(range(R))],
                        ins=[xTsh_d[s][:].opt()],
                        outs=[xTfull_d[s][:].opt()])

        for p in (trps, mlpps, aggps, gph):
            p.release()
        msg_pool.release()
        gmem.release()
        xsb_pool.release()
        ohem.release()

        # ---------------- attention
        a_sb = tc.alloc_tile_pool(name="attn_sb", bufs=1)
        psPrep = tc.alloc_tile_pool(name="psPrep", bufs=2, space="PSUM")

        xT_full_r = {}
        for s in sides:
            sd = sides[s]
            t = a_sb.tile([65, sd["N"]], F32R, name=f"xTfull_{s}")
            NCs = sd["NC"]
            for c in range(R):
                nc.sync.dma_start(t[:, c * NCs:(c + 1) * NCs],
                                  xTfull_d[s][c])
            xT_full_r[s] = t

        xTq_r = {}

        def get_xTq_r(s):
            if s not in xTq_r:
                t = a_sb.tile([65, sides[s]["NC"]], F32R, name=f"xTq_{s}")
                nc.vector.tensor_copy(t[:], xT_cur[s][:])
                xTq_r[s] = t
            return xTq_r[s]

        # heads packed two per tile at partition offsets 0 and 32 (base
        # partitions are restricted to {0, 32, 64})
        dirs = {"mp": ("mol", "prot"), "pm": ("prot", "mol")}
        Wq_r, Wk_r, Wv_r = {}, {}, {}
        for dirn in dirs:
            Wd, bd = dram[f"attn_{dirn}_W"], dram[f"attn_{dirn}_b"]
            wq = [a_sb.tile([65, 64], F32, name=f"wqs_{dirn}{g}")
                  for g in range(2)]
            wk = [a_sb.tile([65, 64], F32, name=f"wks_{dirn}{g}")
                  for g in range(2)]
            wv = a_sb.tile([65, HEADS * 17], F32, name=f"wvs_{dirn}")
            for g in range(2):
                nc.vector.memset(wq[g][:], 0.0)
                nc.vector.memset(wk[g][:], 0.0)
            nc.vector.memset(wv[:], 0.0)
            for h in range(HEADS):
                g, hh = divmod(h, 2)
                # head block rows: 32hh = aug (-m / ones), 32hh+1.. = Q/K
                nc.sync.dma_start(wq[g][0:64, 32 * hh + 1:32 * hh + 17],
                                  Wd[0][:, 16 * h:16 * h + 16])
                nc.sync.dma_start(wq[g][64:65, 32 * hh + 1:32 * hh + 17],
                                  bd[0][None, 16 * h:16 * h + 16])
                nc.sync.dma_start(wk[g][0:64, 32 * hh + 1:32 * hh + 17],
                                  Wd[1][:, 16 * h:16 * h + 16])
                nc.sync.dma_start(wk[g][64:65, 32 * hh + 1:32 * hh + 17],
                                  bd[1][None, 16 * h:16 * h + 16])
                nc.vector.memset(wk[g][64:65, 32 * hh:32 * hh + 1], 1.0)
                nc.sync.dma_start(wv[0:64, 17 * h:17 * h + 16],
                                  Wd[2][:, 16 * h:16 * h + 16])
                nc.sync.dma_start(wv[64:65, 17 * h:17 * h + 16],
                                  bd[2][None, 16 * h:16 * h + 16])
                nc.vector.memset(wv[64:65, 17 * h + 16:17 * h + 17], 1.0)
            Wq_r[dirn] = []
            Wk_r[dirn] = []
            for g in range(2):
                tq = a_sb.tile([65, 64], F32R, name=f"wqr_{dirn}{g}")
                nc.vector.tensor_copy(tq[:], wq[g][:])
                Wq_r[dirn].append(tq)
                tk = a_sb.tile([65, 64], F32R, name=f"wkr_{dirn}{g}")
                nc.vector.tensor_copy(tk[:], wk[g][:])
                Wk_r[dirn].append(tk)
            tv = a_sb.tile([65, HEADS * 17], F32R, name=f"wvr_{dirn}")
            nc.vector.tensor_copy(tv[:], wv[:])
            Wv_r[dirn] = tv

        QTs, KTa, Vp = {}, {}, {}
        prep_order = ("pm", "mp") if KSTAGE >= 1 else ()
        for dirn in prep_order:
            qs, ks = dirs[dirn]
            # K-side first: mp's K side (prot) is ready before mol finishes
            NCq, Nk = sides[qs]["NC"], sides[ks]["N"]
            n_k128 = Nk // 128
            KTa[dirn] = []
            for g in range(2):
                kta = a_sb.tile([64, Nk], F32R, name=f"KTa_{dirn}{g}")
                for c0 in range(0, Nk, 512):
                    psk = psPrep.tile([128, 512], F32, name="prep_ps")
                    nc.tensor.matmul(psk[0:64, :], Wk_r[dirn][g][:],
                                     xT_full_r[ks][:, c0:c0 + 512],
                                     start=True, stop=True)
                    copy_alt(kta[:, c0:c0 + 512], psk[0:64, :])
                KTa[dirn].append(kta)
            vp = a_sb.tile([128, n_k128, HEADS * 17], F32R, name=f"Vp_{dirn}")
            for kt in range(n_k128):
                psv = psPrep.tile([128, 512], F32, name="prep_ps")
                nc.tensor.matmul(psv[:, 0:HEADS * 17],
                                 xT_full_r[ks][:, kt * 128:(kt + 1) * 128],
                                 Wv_r[dirn][:], start=True, stop=True)
                copy_alt(vp[:, kt, :], psv[:, 0:HEADS * 17])
            Vp[dirn] = vp
            QTs[dirn] = []
            for g in range(2):
                psq = psPrep.tile([128, 512], F32, name="prep_ps")
                nc.tensor.matmul(psq[0:64, 0:NCq], Wq_r[dirn][g][:],
                                 get_xTq_r(qs)[:], start=True, stop=True)
                qt_ = a_sb.tile([64, NCq], F32R, name=f"QTs_{dirn}{g}")
                nc.scalar.activation(qt_[:], psq[0:64, 0:NCq], AF.Copy,
                                     scale=0.25)
                QTs[dirn].append(qt_)
        psPrep.release()

        # interleaved pass1 (row max) / pass2 (exp + wV) chains per
        # (direction, head): chain i+2's pass1 (PE+DVE) hides under chain
        # i's pass2 (ACT-bound)
        psS1 = tc.alloc_tile_pool(name="psS1", bufs=2, space="PSUM")
        psT1 = tc.alloc_tile_pool(name="psT1", bufs=1, space="PSUM")
        psS2 = tc.alloc_tile_pool(name="psS2", bufs=2, space="PSUM")
        psOT = tc.alloc_tile_pool(name="psOT", bufs=2, space="PSUM")
        psTr = tc.alloc_tile_pool(name="psTr", bufs=1, space="PSUM")
        ex_pool = tc.alloc_tile_pool(name="expool", bufs=8)

        H_sb = {}
        for dirn, (qs, ks) in dirs.items():
            H_sb[dirn] = a_sb.tile([128, sides[qs]["NC"] // 128, 64], F32,
                                   name=f"H_{dirn}")

        def p1_emit(dirn, h):
            qs, ks = dirs[dirn]
            NCq, Nk = sides[qs]["NC"], sides[ks]["N"]
            n_qt, nch = NCq // 128, Nk // 512
            g_, hh = divmod(h, 2)
            for qt in range(n_qt):
                parts = a_sb.tile([128, 8], F32, name="mxp", bufs=4,
                                  tag="mxp")
                for g in range(nch):
                    ps = psS1.tile([128, 512], F32, name="s1_ps")
                    nc.tensor.matmul(
                        ps[:],
                        QTs[dirn][g_][32 * hh:32 * hh + 17,
                                      qt * 128:(qt + 1) * 128],
                        KTa[dirn][g_][32 * hh:32 * hh + 17,
                                      g * 512:(g + 1) * 512],
                        start=True, stop=True)
                    nc.vector.reduce_max(parts[:, g:g + 1], ps[:], axis=AX.X)
                mq = a_sb.tile([128, 1], F32, name="mq", bufs=4, tag="mq")
                nc.vector.reduce_max(mq[:], parts[:, 0:nch], axis=AX.X)
                mneg = a_sb.tile([128, 1], F32, name="mneg", bufs=4,
                                 tag="mneg")
                nc.vector.tensor_scalar_mul(mneg[:], mq[:], -1.0)
                tp = psT1.tile([1, 128], F32, name="mT_ps")
                nc.tensor.transpose(tp[:], mneg[:], ident_sb[:])
                nc.vector.tensor_copy(
                    QTs[dirn][g_][32 * hh:32 * hh + 1,
                                  qt * 128:(qt + 1) * 128], tp[:])

        def p2_emit(dirn, h):
            qs, ks = dirs[dirn]
            NCq, Nk = sides[qs]["NC"], sides[ks]["N"]
            n_qt, n_k128 = NCq // 128, Nk // 128
            g_, hh = divmod(h, 2)
            H = H_sb[dirn]
            oT = psOT.tile([17, 512], F32, name="oT_ps")
            pair = 2 if NCq <= 256 else 1  # k-chunks sharing one exp slice
            pend = []  # (kc, ex, col): wV lags one step behind s2/exp
            for kc0 in range(0, n_k128, pair):
                s2 = psS2.tile([128, 512], F32, name="s2_ps")
                cur = []
                for j in range(pair):
                    kc = kc0 + j
                    nc.tensor.matmul(
                        s2[:, j * NCq:(j + 1) * NCq],
                        KTa[dirn][g_][32 * hh:32 * hh + 17,
                                      kc * 128:(kc + 1) * 128],
                        QTs[dirn][g_][32 * hh:32 * hh + 17, :],
                        start=True, stop=True)
                ex = ex_pool.tile([128, 512], F32R, name="ex", tag="ex")
                nc.scalar.activation(ex[:, 0:pair * NCq],
                                     s2[:, 0:pair * NCq], AF.Exp)
                for j in range(pair):
                    cur.append((kc0 + j, ex, j * NCq))
                for kc, ext, col in pend:
                    nc.tensor.matmul(
                        oT[:, 0:NCq],
                        Vp[dirn][:, kc, 17 * h:17 * h + 17],
                        ext[:, col:col + NCq], start=(kc == 0), stop=False)
                pend = cur
            for kc, ext, col in pend:
                nc.tensor.matmul(
                    oT[:, 0:NCq], Vp[dirn][:, kc, 17 * h:17 * h + 17],
                    ext[:, col:col + NCq], start=(kc == 0),
                    stop=(kc == n_k128 - 1))
            oT_sb = a_sb.tile([17, NCq], F32, name="oTsb", bufs=2, tag="oTsb")
            copy_alt(oT_sb[:], oT[:, 0:NCq])
            for qt in range(n_qt):
                tp = psTr.tile([128, 17], F32, name="oTr_ps")
                nc.tensor.transpose(tp[:], oT_sb[:, qt * 128:(qt + 1) * 128],
                                    ident_sb[0:17, 0:17])
                inv = a_sb.tile([128, 1], F32, name="inv", bufs=2, tag="inv")
                nc.vector.reciprocal(inv[:], tp[:, 16:17])
                nc.vector.tensor_scalar_mul(
                    H[:, qt, 16 * h:16 * (h + 1)], tp[:, 0:16], inv[:])

        if KSTAGE >= 2:
            chains = [(d, h) for d in ("pm", "mp") for h in range(HEADS)]
            p1_emit(*chains[0])
            p1_emit(*chains[1])
            for i, c in enumerate(chains):
                if i + 2 < len(chains):
                    p1_emit(*chains[i + 2])
                p2_emit(*c)
            for dirn, (qs, ks) in dirs.items():
                nc.vector.tensor_add(H_sb[dirn][:], H_sb[dirn][:],
                                     xnf_f32[qs][:])
        else:
            for dirn, (qs, ks) in dirs.items():
                nc.vector.tensor_copy(H_sb[dirn][:], xnf_f32[qs][:])

        # ---------------- pooling + output MLP
        for dirn, qs in (("mp", "mol"), ("pm", "prot")):
            n_qt = sides[qs]["NC"] // 128
            psz = psS2.tile([128, 512], F32, name="s2_ps")
            for qt in range(n_qt):
                nc.tensor.matmul(psz[0:64, 0:B], H_sb[dirn][:, qt, :],
                                 sb_pmat[qs][:, qt, :],
                                 start=(qt == 0), stop=(qt == n_qt - 1))
            zpart = a_sb.tile([64, B], F32, name=f"zpart_{dirn}")
            nc.vector.tensor_copy(zpart[:], psz[0:64, 0:B])
            row0 = 0 if dirn == "mp" else 64
            nc.sync.dma_start(zt_part_d[row0:row0 + 64, :], zpart[:])
        nc.gpsimd.collective_compute(
            "AllReduce", ALU.add, replica_groups=[list(range(R))],
            ins=[zt_part_d[:].opt()], outs=[zt_full_d[:].opt()])
        zT = a_sb.tile([128, B], F32, name="zT")
        nc.sync.dma_start(zT[:], zt_full_d[:])

        fc1W = a_sb.tile([128, 64], F32, name="fc1W")
        nc.sync.dma_start(fc1W[:], dram["fc1_W"][:])
        fc1b = a_sb.tile([64, 1], F32, name="fc1b")
        nc.sync.dma_start(fc1b[:], dram["fc1_b"][:, None])
        fc2W = a_sb.tile([64, 1], F32, name="fc2W")
        nc.sync.dma_start(fc2W[:], dram["fc2_W"][:])
        fc2b = a_sb.tile([1, 1], F32, name="fc2b")
        nc.sync.dma_start(fc2b[:], dram["fc2_b"][:, None])

        ps = psS2.tile([128, 512], F32, name="s2_ps")
        nc.tensor.matmul(ps[0:64, 0:B], fc1W[:], zT[:], start=True, stop=True)
        h1 = a_sb.tile([65, B], F32, name="h1")
        nc.scalar.activation(h1[0:64, :], ps[0:64, 0:B], AF.Relu, bias=fc1b[:])
        ps2 = psS2.tile([128, 512], F32, name="s2_ps")
        nc.tensor.matmul(ps2[0:1, 0:B], fc2W[:], h1[0:64, :],
                         start=True, stop=True)
        osb = a_sb.tile([1, B], F32, name="osb")
        nc.scalar.activation(osb[:], ps2[0:1, 0:B], AF.Sigmoid, bias=fc2b[:])
        nc.sync.dma_start(out_d[:], osb[:])

        ex_pool.release()
        for p in (psTr, psOT, psS2, psT1, psS1):
            p.release()
        a_sb.release()
        xnf_pool.release()
        xT_pool.release()
        dpool.release()
        const.release()

    nc.compile()
    return nc


# ----------------------------------------------------------------- entry

def kernel(**inputs):
    global last_results
    meta, percore = _prep_host(inputs)
    key = (meta["mol_tile_win"], meta["mol_tile_blk"],
           meta["prot_tile_win"], meta["prot_tile_blk"])
    if key not in _CACHE:
        _CACHE[key] = _build(meta)
    nc = _CACHE[key]
    from concourse.bass_utils import run_bass_kernel_spmd
    res = run_bass_kernel_spmd(nc, percore, list(range(R)))
    last_results = res
    return np.asarray(res.results[0]["out"], np.float32).reshape(B)
